# revision 6
# baseline (speedup 1.0000x reference)
"""MQA attention block (B=2, N=2048, DIM=768, H=12, D=64) on 8 TRN2 NeuronCores.

Sharding: batch x query-block data parallel — core c handles batch c//4,
query rows (c%4)*512..+512. Each core computes K/V for its batch locally
(redundant but cheap), all 12 heads for its query block. No collectives.

Device computes Q/K/V projections, scores, exp, and the un-normalized
AV accumulation (plus row sums via a ones column). All 12 heads' [65, 512]
(64 AV rows + 1 sums row) tiles ship to the host, which normalizes and
applies the output projection + bias (cheap: one [512,768]x[768,768] GEMM
per core).

Orientation: all tensors flow "transposed" (channels on partitions):
  QT[c,i] = Wq.T-proj, K2T[d,j] (duplicated to both partition halves),
  ST[j,i] scores -> exp on ACT -> AV via V_ext=[V|ones] giving out^T and
  row sums in one matmul.
"""

import sys

for _p in ("/opt/trn_rl_repo",):
    if _p not in sys.path:
        sys.path.insert(0, _p)

import numpy as np
import ml_dtypes

BF = ml_dtypes.bfloat16

B, N, DIM = 2, 2048, 768
H, D = 12, 64
NQ = 512            # query rows per core
SCALE = D ** -0.5
NCORES = 8
FT = DIM // 128     # 6 partition tiles of the channel dim
JT = N // 128       # 16 key tiles
NJ = N // 512       # 4


def _patch_tile_drain(tile_mod):
    """This toolchain snapshot rejects >1 sync-wait per instruction at walrus
    codegen, but TileContext's tail drain stacks every outstanding sem wait
    onto a single Drain. Split them: one drain instruction per wait."""
    import bass_rust
    from concourse.vector_clock import ScopedClock

    def _drain_and_barrier(self, tick_clock, wait_clock):
        nc = self.nc
        drain_inst = nc.sync.drain()
        wait_clock.add_sem_waits(
            drain_inst.ins, ScopedClock({None: tick_clock.global_clock})
        )
        waits = list(drain_inst.ins.sync_info.on_wait)
        if len(waits) > 1:
            drain_inst.ins.sync_info = bass_rust.SyncInfo(
                on_wait=[waits[0]], on_update=[]
            )
            for w in waits[1:]:
                extra = nc.sync.drain()
                extra.ins.sync_info = bass_rust.SyncInfo(on_wait=[w], on_update=[])
        nc.all_engine_barrier()
        assert self.sems is not None
        popped = nc._tile_sem_poison_stack.pop()
        assert popped is self._sem_poison
        nc.clear_and_free_semaphores(list(self.sems.allocated().values()))

    tile_mod.TileContext._drain_and_barrier = _drain_and_barrier


def _split_multi_waits(nc):
    """Same toolchain limitation, applied globally: walrus rejects any
    instruction carrying >1 sync-wait. Move extra waits onto fresh NoOps
    inserted just before the instruction on the same engine (engine streams
    are in-order, so this is semantically identical)."""
    from concourse import mybir

    n = 0
    for f in nc.m.functions:
        for bb in f.blocks:
            insts = bb.instructions
            out = []
            for inst in insts:
                si = inst.sync_info
                waits = list(si.on_wait) if si is not None else []
                if len(waits) > 1:
                    for w in waits[:-1]:
                        n += 1
                        out.append(
                            mybir.InstNoOp(
                                name=f"waitsplit_{n}",
                                engine=inst.engine,
                                sync_info=mybir.SyncInfo(on_wait=[w], on_update=[]),
                                bass_nofuse=True,
                            )
                        )
                    inst.sync_info = mybir.SyncInfo(
                        on_wait=[waits[-1]], on_update=list(si.on_update)
                    )
                out.append(inst)
            insts[:] = out


def build_graph():
    import concourse.bass as bass
    import concourse.tile as tile
    from concourse import mybir

    _patch_tile_drain(tile)

    f32 = mybir.dt.float32
    bf16 = mybir.dt.bfloat16
    i16 = mybir.dt.int16
    fp8 = mybir.dt.float8e4
    DR = mybir.MatmulPerfMode.DoubleRow
    EXP = mybir.ActivationFunctionType.Exp
    MUL = mybir.AluOpType.mult
    ADD = mybir.AluOpType.add

    # DVE-side softmax: Schraudolph integer exp directly into bf16 bit
    # patterns: e^s ~= bitcast_bf16(int16(round(s * 128/ln2 + (127*128 - C)))).
    # ~2% rms multiplicative error that largely cancels in the softmax
    # normalization (sums use the same approximated e). Offloads ~40% of the
    # exp stream from the saturated ACT engine onto the otherwise-idle DVE.
    A_SCH = float(2**7 / np.log(2))
    B_SCH = float(127 * 2**7 - 6.0)

    def dve_pick(t, j):
        # ACT alone paces the kernel at ~1147ns/j; handing odd js of the
        # steady pairs to the DVE (1192ns/j, off the critical ACT stream)
        # drops the softmax cadence below the PE's ~700ns/j.
        if t == 0:
            return False
        return j % 2 == 1 and j < 15

    nc = bass.Bass()
    # all inputs arrive as exact SBUF images ([partition, free] layout built
    # on host) so each loads with one large-descriptor DMA.
    # xt image free layout: half*6144 + ft*1024 + col  (halves outer)
    xT_e = nc.declare_dram_parameter("xT", [128, FT * N], bf16, isOutput=False)
    # w0 = [wkv image | wq ct=0 piece] in one tensor: loads as a single
    # wide DMA (per-partition-line setup cost dominates small transfers)
    w0_e = nc.declare_dram_parameter("w0", [128, FT * 2 * D + DIM], bf16, isOutput=False)
    wq_e = nc.declare_dram_parameter("wq", [128, (FT - 1) * DIM], bf16, isOutput=False)
    # bf16 outputs: halves the drain-DMA bytes and doubles the cast rate;
    # ~0.4% quantization on AV+sums is far inside the error budget
    st_e = [
        nc.declare_dram_parameter(f"st{h}", [65, NQ], bf16, isOutput=True)
        for h in range(H)
    ]

    with tile.TileContext(nc) as tc:
        with (
            tc.tile_pool(name="persist", bufs=1) as P,
            tc.tile_pool(name="work", bufs=2) as W,
            tc.tile_pool(name="psum", bufs=2, space="PSUM") as PS,
        ):
            # ---------------- input loads (one DMA per tensor) -----------
            # Each logical [768, x] tensor lands as one [128, 6*x] SBUF tile
            # (f-tile ft at columns ft*x:(ft+1)*x) via a single 3D-AP DMA —
            # the ~0.6us per-dma_start sequencer issue cost dominates loads
            # otherwise. xT arrives np.roll'd per core so the query block is
            # always columns 0:NQ (softmax is key-permutation invariant).
            xt = P.tile([128, FT * N], bf16, tag="xt", name="xt")
            w0s = P.tile([128, FT * 2 * D + DIM], bf16, tag="w0s", name="w0s")
            wqs = P.tile([128, (FT - 1) * DIM], bf16, tag="wqs", name="wqs")

            def xTs(ft, sl):
                a, b_ = sl.start or 0, sl.stop
                q = a // 512
                assert (b_ - 1) // 512 == q
                base = q * 3072 + ft * 512
                return xt[:, base + a - q * 512 : base + b_ - q * 512]

            # Aggregate inbound DMA BW is ~140GB/s shared across the three
            # DMA-capable queues. Chunk 0 (queries + first keys) gates the
            # whole exp stream, so it loads as six ft-ordered pieces round-
            # robined over sync/gpsimd/scalar — the K projection consumes
            # them in arrival order. wq's ct=0 piece follows split in two;
            # later chunks ride sync/gpsimd and later wq cts ride scalar,
            # each landing just before its consumer.
            # Per-queue DMA throughput is dominated by a fixed per-
            # partition-line cost, so fewer/wider transfers win: x loads as
            # four whole chunks alternating sync/gpsimd (chunk c lands just
            # before its K-projection consumers), weights as two wide
            # transfers on the otherwise-idle scalar queue.
            # sync + scalar are hardware-DGE queues (~2x the throughput of
            # gpsimd's software DGE), so all input loads ride those two;
            # gpsimd only carries output DMAs later.
            nc.scalar.dma_start(out=w0s, in_=w0_e[:, :])
            nc.sync.dma_start(out=xt[:, 0:3072], in_=xT_e[:, 0:3072])
            nc.scalar.dma_start(out=xt[:, 3072:6144], in_=xT_e[:, 3072:6144])
            nc.sync.dma_start(out=xt[:, 6144:9216], in_=xT_e[:, 6144:9216])
            nc.sync.dma_start(out=xt[:, 9216:12288], in_=xT_e[:, 9216:12288])
            nc.scalar.dma_start(out=wqs[:, 0:768], in_=wq_e[:, 0:768])
            nc.scalar.dma_start(out=wqs[:, 768:3840], in_=wq_e[:, 768:3840])

            # ---------------- PE pre-warm -------------------
            # ~15 junk matmuls during the input-DMA wait push the PE past the
            # HAM activity window so K(0)/Q(0) run at 2.4GHz instead of 1.2.
            junk = P.tile([128, 512], bf16, tag="junk", name="junk")
            nc.vector.memset(junk, 0.5)
            warm_ps = PS.tile([128, 512], f32, tag="av", name="warm_ps", bufs=4)
            # coarse warm-up while the chunk-0 DMA is in flight, then a
            # short-matmul tail so K-A's start quantizes at ~290ns (cold
            # N=128) instead of ~630ns (cold N=512) against the arrival
            for i in range(7):
                nc.tensor.matmul(
                    warm_ps,
                    lhsT=junk[:, 0:128],
                    rhs=junk,
                    start=(i == 0),
                    stop=False,
                )
            for i in range(5):
                nc.tensor.matmul(
                    warm_ps[:, 0:128],
                    lhsT=junk[:, 0:128],
                    rhs=junk[:, 0:128],
                    start=False,
                    stop=(i == 4),
                )
            warm_out = P.tile([128, 16], f32, tag="warm_out", name="warm_out")
            nc.vector.tensor_copy(warm_out, warm_ps[:, 0:16])

            # ---------------- Q^T projection ----------------
            # qt[t] holds heads 2t (partitions 0:64) and 2t+1 (64:128).
            qt = [P.tile([128, NQ], bf16, tag=f"qt{t}", name=f"qt{t}") for t in range(FT)]

            qps = {}

            def emit_q_part(ct, fts, done):
                if ct not in qps:
                    qps[ct] = PS.tile([128, NQ], f32, tag="av", name="ps_q", bufs=4)
                ps_q = qps[ct]
                base = FT * 2 * D if ct == 0 else (ct - 1) * DIM
                wsrc = w0s if ct == 0 else wqs
                for ft in fts:
                    nc.tensor.matmul(
                        ps_q,
                        lhsT=wsrc[:, base + ft * 128 : base + (ft + 1) * 128],
                        rhs=xTs(ft, slice(0, NQ)),
                        start=(ft == 0),
                        stop=(ft == FT - 1),
                    )
                if done:
                    nc.vector.tensor_copy(qt[ct], qps.pop(ct))

            def emit_q(ct):
                emit_q_part(ct, range(FT), True)

            # ---------------- attention emitters --------------------------
            # Per pair t: heads a=2t (partitions 0:64 of qt[t]) and b=2t+1
            # (64:128). Per j: two S matmuls (row groups 0/64) into one
            # [128, 2, 512] psum tile, one exp for both; AV matmuls
            # (lhsT=[V|ones] -> psum rows 0:64 out^T + row 64 sums) trail
            # the exps by 1 (head a) / 2 (head b). AV psums stage to SBUF
            # at the pair tail and DMA to DRAM; host normalizes + projects.
            # e tiles are allocated int16 with a bf16 bitcast view over the
            # same bytes: ACT writes exp() through the bf16 view, the DVE
            # writes Schraudolph int16 bit patterns natively, and the AV
            # matmuls always read the bf16 view.
            es = [
                [W.tile([128, 2, NQ], i16, tag=f"e{j}", name=f"e{j}", bufs=2) for j in range(JT)]
                for _ in range(2)
            ]
            es_bf = [[e.bitcast(bf16) for e in row] for row in es]
            k2t = P.tile([128, N], bf16, tag="k2t", name="k2t")
            vext = [P.tile([128, 128], bf16, tag=f"v{j}", name=f"v{j}") for j in range(JT)]
            avps = {}

            def emit_v(j):
                nc.vector.memset(vext[j][:, D:128], 0.0)
                nc.vector.memset(vext[j][:, D : D + 1], 1.0)
                ps_v = PS.tile([128, D], f32, tag="av", name="ps_v", bufs=4)
                for ft in range(FT):
                    nc.tensor.matmul(
                        ps_v,
                        lhsT=xTs(ft, slice(j * 128, (j + 1) * 128)),
                        rhs=w0s[:, ft * 2 * D + D : ft * 2 * D + 2 * D],
                        start=(ft == 0),
                        stop=(ft == FT - 1),
                    )
                nc.vector.tensor_copy(vext[j][:, 0:D], ps_v)

            def emit_av(t, j, head):
                ps_av = avps[t][head]
                nc.tensor.matmul(
                    ps_av,
                    lhsT=vext[j],
                    rhs=es_bf[t % 2][j][:, head, :],
                    start=(j == 0),
                    stop=(j == JT - 1),
                )

            def emit_pair_seg(t, j_lo, j_hi, pre=None):
                e = es[t % 2]
                e_bf = es_bf[t % 2]
                if j_lo == 0:
                    avps[t] = (
                        PS.tile([128, NQ], f32, tag="av", name="av_a", bufs=4),
                        PS.tile([128, NQ], f32, tag="av", name="av_b", bufs=4),
                    )
                for j in range(j_lo, j_hi):
                    ps_s = PS.tile([128, 2, NQ], f32, tag="s", name="s", bufs=2)
                    nc.tensor.matmul(
                        ps_s[:, 0, :],
                        lhsT=k2t[0:64, j * 128 : (j + 1) * 128],
                        rhs=qt[t][0:64, :],
                        start=True,
                        stop=True,
                    )
                    nc.tensor.matmul(
                        ps_s[:, 1, :],
                        lhsT=k2t[64:128, j * 128 : (j + 1) * 128],
                        rhs=qt[t][64:128, :],
                        start=True,
                        stop=True,
                    )
                    if dve_pick(t, j):
                        nc.vector.tensor_scalar(
                            out=e[j],
                            in0=ps_s,
                            scalar1=A_SCH,
                            scalar2=B_SCH,
                            op0=MUL,
                            op1=ADD,
                        )
                    else:
                        nc.scalar.activation(out=e_bf[j], in_=ps_s, func=EXP)
                    if pre is not None:
                        pre(j)
                    # pair 0 trails its AVs two extra js: its earliest
                    # iterations carry the K-B/c1 projection chain and are
                    # the tightest against the exp cadence
                    la, lb = (4, 5) if t == 0 else (1, 2)
                    if j >= la and (t != 0 or j - la <= 10):
                        emit_av(t, j - la, 0)
                    if j >= lb and (t != 0 or j - lb <= 9):
                        emit_av(t, j - lb, 1)
                    if 1 <= t <= 4:
                        if j == 10:
                            emit_q_part(t + 1, range(3), False)
                        if j == 12:
                            emit_q_part(t + 1, range(3, FT), True)

            def emit_pair_tail(t):
                last = t == H // 2 - 1
                ps_av_a, ps_av_b = avps[t]
                emit_av(t, JT - 2, 1)
                emit_av(t, JT - 1, 0)
                sta = W.tile([65, NQ], bf16, tag="sta", name="sta", bufs=3)
                nc.vector.tensor_copy(sta, ps_av_a[0:65, :])
                eng_a = nc.gpsimd if t % 2 == 0 else nc.sync
                eng_a.dma_start(out=st_e[2 * t][:, :], in_=sta)
                emit_av(t, JT - 1, 1)
                avps.pop(t)
                stb = W.tile([65, NQ], bf16, tag="stb", name="stb", bufs=3)
                if last:
                    # scalar engine is idle once the final exp retires: do the
                    # drain copy AND the DMA there, parallel to sta's path
                    nc.scalar.copy(stb, ps_av_b[0:65, :])
                    nc.scalar.dma_start(out=st_e[2 * t + 1][:, :], in_=stb)
                else:
                    nc.vector.tensor_copy(stb, ps_av_b[0:65, :])
                    eng_b = nc.sync if t % 2 == 0 else nc.gpsimd
                    eng_b.dma_start(out=st_e[2 * t + 1][:, :], in_=stb)

            # ---------------- K^T proj + pair 0, chunk-pipelined ----------
            # K2T[d, j]: K^T computed twice via col-tiled dual matmul groups
            # (cols 0:64 / 64:128 run concurrently) -> one [128, 512] psum.
            # V projection is emitted inside pair 0's j loop so the PE
            # stream stays dense while exps drain.
            kps = {}

            def emit_k_part(nj, fts, done, cs=None, ks=None, dup=True):
                cs = cs if cs is not None else slice(nj * 512, (nj + 1) * 512)
                if nj not in kps:
                    kps[nj] = PS.tile([128, 512], f32, tag="av", name="ps_k", bufs=4)
                ps_k = kps[nj]
                if ks is not None:
                    ps_k = ps_k[:, ks]
                for ft in fts:
                    nc.tensor.matmul(
                        ps_k[0:64, :],
                        lhsT=w0s[:, ft * 2 * D : ft * 2 * D + D],
                        rhs=xTs(ft, cs),
                        start=(ft == 0),
                        stop=(ft == FT - 1),
                        skip_group_check=True,
                    )
                    if dup:
                        nc.tensor.matmul(
                            ps_k[64:128, :],
                            lhsT=w0s[:, ft * 2 * D : ft * 2 * D + D],
                            rhs=xTs(ft, cs),
                            start=(ft == 0),
                            stop=(ft == FT - 1),
                            tile_position=(0, 64),
                            skip_group_check=True,
                        )
                if done:
                    src_ps = kps.pop(nj) if ks is None else ps_k
                    if dup:
                        nc.vector.tensor_copy(k2t[:, cs], src_ps)
                    else:
                        nc.vector.tensor_copy(k2t[0:64, cs], src_ps[0:64, :])
                        # head-b S matmuls need K on partitions 64:128 too;
                        # the gpsimd DMA queue is idle until the first output
                        nc.gpsimd.dma_start(
                            out=k2t[64:128, cs], in_=k2t[0:64, cs]
                        )

            def emit_k(nj):
                emit_k_part(nj, range(FT), True)

            # First S matmul needs qt[0] (all of chunk 0) but only keys
            # 0:128 of k2t — project those keys first, then Q0; the rest of
            # chunk 0's keys and V follow inside pair 0's early iterations.
            emit_k_part(0, range(FT), False, cs=slice(0, 128), ks=slice(0, 128))
            nc.vector.tensor_copy(k2t[:, 0:128], kps[0][:, 0:128])
            emit_q(0)

            def pair0_pre(j):
                # finish chunk 0's keys, pace V one tile per j, spread later
                # K chunks across js, and fold Q1 into the stall window
                if j == 0:
                    emit_k_part(
                        0, range(FT), False, cs=slice(128, 512), ks=slice(128, 512)
                    )
                    nc.vector.tensor_copy(k2t[:, 128:512], kps.pop(0)[:, 128:512])
                    emit_v(0)
                if j + 1 < JT:
                    emit_v(j + 1)
                # K-chunk pacing: emit each chunk's matmuls no earlier than
                # its x DMA lands (in-order PE queue — early emission blocks
                # the S stream behind a data wait). c1 and c3 land with no
                # slack for the ~2us dup-DMA latency, so they dup via the
                # dual matmul; c2 has slack and dups via idle-gpsimd DMA.
                if j == 1:
                    emit_k_part(1, range(0, 2), False, dup=True)
                if j == 2:
                    emit_k_part(1, range(2, 4), False, dup=True)
                if j == 3:
                    emit_k_part(1, range(4, FT), True, dup=True)
                    emit_k_part(2, range(3), False, dup=False)
                if j == 4:
                    emit_k_part(2, range(3, FT), True, dup=False)
                if j == 8:
                    emit_k_part(3, range(0, 2), False, dup=True)
                if j == 9:
                    emit_k_part(3, range(2, 4), False, dup=True)
                if j == 10:
                    emit_k_part(3, range(4, FT), True, dup=True)
                if j == 12:
                    emit_q_part(1, range(3), False)
                if j == 14:
                    emit_q_part(1, range(3, FT), True)

            emit_pair_seg(0, 0, JT, pre=pair0_pre)

            # pair 0 is PE-oversubscribed (all projections + V live there),
            # so its last AV matmuls and output drain spill into pair 1's
            # slack; the av psum pool holds both pairs' accumulators (4
            # bufs) until pair 0 drains at pair-1 j4/j5.
            def pair1_pre(j):
                if j <= 4:
                    emit_av(0, 11 + j, 0)
                if j <= 5:
                    emit_av(0, 10 + j, 1)
                if j == 4:
                    sta = W.tile([65, NQ], bf16, tag="sta", name="sta", bufs=3)
                    nc.vector.tensor_copy(sta, avps[0][0][0:65, :])
                    nc.gpsimd.dma_start(out=st_e[0][:, :], in_=sta)
                if j == 5:
                    stb = W.tile([65, NQ], bf16, tag="stb", name="stb", bufs=3)
                    nc.vector.tensor_copy(stb, avps[0][1][0:65, :])
                    nc.sync.dma_start(out=st_e[1][:, :], in_=stb)
                    avps.pop(0)

            emit_pair_seg(1, 0, JT, pre=pair1_pre)
            emit_pair_tail(1)

            # ---------------- remaining pairs -----------------------------
            for t in range(2, H // 2):
                emit_pair_seg(t, 0, JT)
                emit_pair_tail(t)

    _split_multi_waits(nc)
    return nc


def make_in_maps(x, Wq, Wkv, Wproj, bproj):

    def image(a, p=128):
        # [G*p, w] -> [p, G*w] SBUF image (block g at columns g*w:(g+1)*w)
        gp, w = a.shape
        return np.ascontiguousarray(
            a.reshape(gp // p, p, w).transpose(1, 0, 2).reshape(p, -1)
        )

    wq_b = image((Wq * SCALE).astype(BF))
    # regroup to ct-major: piece ct = all six 128-row in-chunks of output
    # columns ct*128:(ct+1)*128, contiguous for piecewise DMA
    wq_b = np.ascontiguousarray(
        wq_b.reshape(128, FT, FT, 128).transpose(0, 2, 1, 3).reshape(128, FT * DIM)
    )
    wkv_b = image(Wkv.astype(BF))
    w0_b = np.ascontiguousarray(np.concatenate([wkv_b, wq_b[:, 0:DIM]], axis=1))
    wqr_b = np.ascontiguousarray(wq_b[:, DIM:])

    xTb = [x[b].T.astype(BF) for b in range(B)]

    in_maps = []
    for c in range(NCORES):
        b, q0 = c // 4, (c % 4) * NQ
        xr = np.roll(xTb[b], -q0, axis=1)  # [768, 2048]
        # image with halves outer: [128, half*6144 + ft*1024 + col]
        xi = (
            xr.reshape(FT, 128, 4, 512)
            .transpose(1, 2, 0, 3)
            .reshape(128, FT * N)
        )
        in_maps.append(
            {
                "xT": np.ascontiguousarray(xi),
                "w0": w0_b,
                "wq": wqr_b,
            }
        )
    return in_maps


def assemble_out(results, Wproj, bproj):
    Wp = Wproj.astype(np.float32)
    bp = bproj.astype(np.float32)
    out = np.empty((B, N, DIM), dtype=np.float32)
    for c in range(NCORES):
        b, q0 = c // 4, (c % 4) * NQ
        o = np.empty((NQ, DIM), dtype=np.float32)
        for h in range(H):
            st = results[c][f"st{h}"].astype(np.float32)
            o[:, h * D : (h + 1) * D] = (st[0:D] / st[D : D + 1]).T
        out[b, q0 : q0 + NQ, :] = o @ Wp + bp
    return out


def kernel(x, Wq, Wkv, Wproj, bproj, num_layer=None):
    from concourse.bass_utils import run_bass_kernel_spmd

    x = np.asarray(x, dtype=np.float32)
    Wq = np.asarray(Wq, dtype=np.float32)
    Wkv = np.asarray(Wkv, dtype=np.float32)
    Wproj = np.asarray(Wproj, dtype=np.float32)
    bproj = np.asarray(bproj, dtype=np.float32)

    in_maps = make_in_maps(x, Wq, Wkv, Wproj, bproj)
    nc = build_graph()
    res = run_bass_kernel_spmd(nc, in_maps, core_ids=list(range(NCORES)))
    return assemble_out(res.results, Wproj, bproj)



# revision 14
# speedup vs baseline: 1.0551x; 1.0551x over previous
"""MQA attention block (B=2, N=2048, DIM=768, H=12, D=64) on 8 TRN2 NeuronCores.

Sharding: batch x query-block data parallel — core c handles batch c//4,
query rows (c%4)*512..+512. Each core computes K/V for its batch locally
(redundant but cheap), all 12 heads for its query block. No collectives.

Device computes Q/K/V projections, scores, exp, and the un-normalized
AV accumulation (plus row sums via a ones column). All 12 heads' [65, 512]
(64 AV rows + 1 sums row) tiles ship to the host, which normalizes and
applies the output projection + bias (cheap: one [512,768]x[768,768] GEMM
per core).

Orientation: all tensors flow "transposed" (channels on partitions):
  QT[c,i] = Wq.T-proj, K2T[d,j] (duplicated to both partition halves),
  ST[j,i] scores -> exp on ACT -> AV via V_ext=[V|ones] giving out^T and
  row sums in one matmul.
"""

import sys

for _p in ("/opt/trn_rl_repo",):
    if _p not in sys.path:
        sys.path.insert(0, _p)

import numpy as np
import ml_dtypes

BF = ml_dtypes.bfloat16

B, N, DIM = 2, 2048, 768
H, D = 12, 64
NQ = 512            # query rows per core
SCALE = D ** -0.5
NCORES = 8
FT = DIM // 128     # 6 partition tiles of the channel dim
JT = N // 128       # 16 key tiles
NJ = N // 512       # 4


def _patch_tile_drain(tile_mod):
    """This toolchain snapshot rejects >1 sync-wait per instruction at walrus
    codegen, but TileContext's tail drain stacks every outstanding sem wait
    onto a single Drain. Split them: one drain instruction per wait."""
    import bass_rust
    from concourse.vector_clock import ScopedClock

    def _drain_and_barrier(self, tick_clock, wait_clock):
        nc = self.nc
        drain_inst = nc.sync.drain()
        wait_clock.add_sem_waits(
            drain_inst.ins, ScopedClock({None: tick_clock.global_clock})
        )
        waits = list(drain_inst.ins.sync_info.on_wait)
        if len(waits) > 1:
            drain_inst.ins.sync_info = bass_rust.SyncInfo(
                on_wait=[waits[0]], on_update=[]
            )
            for w in waits[1:]:
                extra = nc.sync.drain()
                extra.ins.sync_info = bass_rust.SyncInfo(on_wait=[w], on_update=[])
        nc.all_engine_barrier()
        assert self.sems is not None
        popped = nc._tile_sem_poison_stack.pop()
        assert popped is self._sem_poison
        nc.clear_and_free_semaphores(list(self.sems.allocated().values()))

    tile_mod.TileContext._drain_and_barrier = _drain_and_barrier


def _split_multi_waits(nc):
    """Same toolchain limitation, applied globally: walrus rejects any
    instruction carrying >1 sync-wait. Move extra waits onto fresh NoOps
    inserted just before the instruction on the same engine (engine streams
    are in-order, so this is semantically identical)."""
    from concourse import mybir

    n = 0
    for f in nc.m.functions:
        for bb in f.blocks:
            insts = bb.instructions
            out = []
            for inst in insts:
                si = inst.sync_info
                waits = list(si.on_wait) if si is not None else []
                if len(waits) > 1:
                    for w in waits[:-1]:
                        n += 1
                        out.append(
                            mybir.InstNoOp(
                                name=f"waitsplit_{n}",
                                engine=inst.engine,
                                sync_info=mybir.SyncInfo(on_wait=[w], on_update=[]),
                                bass_nofuse=True,
                            )
                        )
                    inst.sync_info = mybir.SyncInfo(
                        on_wait=[waits[-1]], on_update=list(si.on_update)
                    )
                out.append(inst)
            insts[:] = out


def build_graph():
    import concourse.bass as bass
    import concourse.tile as tile
    from concourse import mybir

    _patch_tile_drain(tile)

    f32 = mybir.dt.float32
    bf16 = mybir.dt.bfloat16
    i16 = mybir.dt.int16
    fp8 = mybir.dt.float8e4
    DR = mybir.MatmulPerfMode.DoubleRow
    EXP = mybir.ActivationFunctionType.Exp
    MUL = mybir.AluOpType.mult
    ADD = mybir.AluOpType.add

    # DVE-side softmax: Schraudolph integer exp directly into bf16 bit
    # patterns: e^s ~= bitcast_bf16(int16(round(s * 128/ln2 + (127*128 - C)))).
    # ~2% rms multiplicative error that largely cancels in the softmax
    # normalization (sums use the same approximated e). Offloads ~40% of the
    # exp stream from the saturated ACT engine onto the otherwise-idle DVE.
    A_SCH = float(2**7 / np.log(2))
    B_SCH = float(127 * 2**7 - 6.0)

    def dve_pick(t, j):
        # ACT alone paces the kernel at ~1147ns/j; handing odd js of the
        # steady pairs to the DVE (1192ns/j, off the critical ACT stream)
        # drops the softmax cadence below the PE's ~700ns/j. Pair 0's DVE is
        # loaded with k2t/vext/qt evacuations, so it only takes late js.
        if t == 0:
            return j in (9, 11, 13)
        return j % 2 == 1 and j < 15

    nc = bass.Bass()
    # all inputs arrive as exact SBUF images ([partition, free] layout built
    # on host) so each loads with one large-descriptor DMA.
    # xt image free layout: half*6144 + ft*1024 + col  (halves outer)
    xT_e = nc.declare_dram_parameter("xT", [128, FT * N], bf16, isOutput=False)
    # w0 = [wkv image | wq ct=0 piece] in one tensor: loads as a single
    # wide DMA (per-partition-line setup cost dominates small transfers)
    w0_e = nc.declare_dram_parameter("w0", [128, FT * 2 * D + DIM], bf16, isOutput=False)
    wq_e = nc.declare_dram_parameter("wq", [128, (FT - 1) * DIM], bf16, isOutput=False)
    # bf16 outputs: halves the drain-DMA bytes and doubles the cast rate;
    # ~0.4% quantization on AV+sums is far inside the error budget
    st_e = [
        nc.declare_dram_parameter(f"st{h}", [65, NQ], bf16, isOutput=True)
        for h in range(H)
    ]

    with tile.TileContext(nc) as tc:
        with (
            tc.tile_pool(name="persist", bufs=1) as P,
            tc.tile_pool(name="work", bufs=2) as W,
            tc.tile_pool(name="psum", bufs=2, space="PSUM") as PS,
        ):
            # ---------------- input loads (one DMA per tensor) -----------
            # Each logical [768, x] tensor lands as one [128, 6*x] SBUF tile
            # (f-tile ft at columns ft*x:(ft+1)*x) via a single 3D-AP DMA —
            # the ~0.6us per-dma_start sequencer issue cost dominates loads
            # otherwise. xT arrives np.roll'd per core so the query block is
            # always columns 0:NQ (softmax is key-permutation invariant).
            xt = P.tile([128, FT * N], bf16, tag="xt", name="xt")
            w0s = P.tile([128, FT * 2 * D + DIM], bf16, tag="w0s", name="w0s")
            wqs = P.tile([128, (FT - 1) * DIM], bf16, tag="wqs", name="wqs")

            def xTs(ft, sl):
                a, b_ = sl.start or 0, sl.stop
                q = a // 512
                assert (b_ - 1) // 512 == q
                base = q * 3072 + ft * 512
                return xt[:, base + a - q * 512 : base + b_ - q * 512]

            # Aggregate inbound DMA BW is ~140GB/s shared across the three
            # DMA-capable queues. Chunk 0 (queries + first keys) gates the
            # whole exp stream, so it loads as six ft-ordered pieces round-
            # robined over sync/gpsimd/scalar — the K projection consumes
            # them in arrival order. wq's ct=0 piece follows split in two;
            # later chunks ride sync/gpsimd and later wq cts ride scalar,
            # each landing just before its consumer.
            # Per-queue DMA throughput is dominated by a fixed per-
            # partition-line cost, so fewer/wider transfers win: x loads as
            # four whole chunks alternating sync/gpsimd (chunk c lands just
            # before its K-projection consumers), weights as two wide
            # transfers on the otherwise-idle scalar queue.
            # sync + scalar are hardware-DGE queues (~2x the throughput of
            # gpsimd's software DGE), so all input loads ride those two;
            # gpsimd only carries output DMAs later.
            # Arrival order tuned for the exp-stream start: w0 (wkv + wq-ct0)
            # first on scalar, chunk 0 in three ft-pair pieces on sync (the
            # K/Q projections consume pieces as they land), then the later
            # chunks paced to their first S consumers. wq ct1 rides early so
            # pair 1's Q projection never waits.
            nc.scalar.dma_start(out=w0s, in_=w0_e[:, :])
            nc.scalar.dma_start(out=wqs[:, 0:768], in_=wq_e[:, 0:768])
            nc.sync.dma_start(out=xt[:, 0:1024], in_=xT_e[:, 0:1024])
            nc.sync.dma_start(out=xt[:, 1024:2048], in_=xT_e[:, 1024:2048])
            nc.sync.dma_start(out=xt[:, 2048:3072], in_=xT_e[:, 2048:3072])
            nc.sync.dma_start(out=xt[:, 3072:4608], in_=xT_e[:, 3072:4608])
            nc.sync.dma_start(out=xt[:, 4608:6144], in_=xT_e[:, 4608:6144])
            nc.scalar.dma_start(out=xt[:, 6144:9216], in_=xT_e[:, 6144:9216])
            nc.sync.dma_start(out=xt[:, 9216:12288], in_=xT_e[:, 9216:12288])
            nc.scalar.dma_start(out=wqs[:, 768:3840], in_=wq_e[:, 768:3840])

            # ---------------- PE pre-warm -------------------
            # ~15 junk matmuls during the input-DMA wait push the PE past the
            # HAM activity window so K(0)/Q(0) run at 2.4GHz instead of 1.2.
            junk = P.tile([128, 512], bf16, tag="junk", name="junk")
            nc.vector.memset(junk, 0.5)
            warm_ps = PS.tile([128, 512], f32, tag="av", name="warm_ps", bufs=4)
            # coarse warm-up while the chunk-0 DMA is in flight, then a
            # short-matmul tail so K-A's start quantizes at ~290ns (cold
            # N=128) instead of ~630ns (cold N=512) against the arrival
            for i in range(7):
                nc.tensor.matmul(
                    warm_ps,
                    lhsT=junk[:, 0:128],
                    rhs=junk,
                    start=(i == 0),
                    stop=False,
                )
            for i in range(5):
                nc.tensor.matmul(
                    warm_ps[:, 0:128],
                    lhsT=junk[:, 0:128],
                    rhs=junk[:, 0:128],
                    start=False,
                    stop=(i == 4),
                )
            warm_out = P.tile([128, 16], f32, tag="warm_out", name="warm_out")
            nc.vector.tensor_copy(warm_out, warm_ps[:, 0:16])

            # ---------------- Q^T projection ----------------
            # qt[t] holds heads 2t (partitions 0:64) and 2t+1 (64:128).
            qt = [P.tile([128, NQ], bf16, tag=f"qt{t}", name=f"qt{t}") for t in range(FT)]

            qps = {}

            def emit_q_part(ct, fts, done):
                if ct not in qps:
                    qps[ct] = PS.tile([128, NQ], f32, tag="av", name="ps_q", bufs=4)
                ps_q = qps[ct]
                base = FT * 2 * D if ct == 0 else (ct - 1) * DIM
                wsrc = w0s if ct == 0 else wqs
                for ft in fts:
                    nc.tensor.matmul(
                        ps_q,
                        lhsT=wsrc[:, base + ft * 128 : base + (ft + 1) * 128],
                        rhs=xTs(ft, slice(0, NQ)),
                        start=(ft == 0),
                        stop=(ft == FT - 1),
                    )
                if done:
                    nc.vector.tensor_copy(qt[ct], qps.pop(ct))

            def emit_q(ct):
                emit_q_part(ct, range(FT), True)

            # ---------------- attention emitters --------------------------
            # Per pair t: heads a=2t (partitions 0:64 of qt[t]) and b=2t+1
            # (64:128). Per j: two S matmuls (row groups 0/64) into one
            # [128, 2, 512] psum tile, one exp for both; AV matmuls
            # (lhsT=[V|ones] -> psum rows 0:64 out^T + row 64 sums) trail
            # the exps by 1 (head a) / 2 (head b). AV psums stage to SBUF
            # at the pair tail and DMA to DRAM; host normalizes + projects.
            # e tiles are allocated int16 with a bf16 bitcast view over the
            # same bytes: ACT writes exp() through the bf16 view, the DVE
            # writes Schraudolph int16 bit patterns natively, and the AV
            # matmuls always read the bf16 view.
            es = [
                [W.tile([128, 2, NQ], i16, tag=f"e{j}", name=f"e{j}", bufs=2) for j in range(JT)]
                for _ in range(2)
            ]
            es_bf = [[e.bitcast(bf16) for e in row] for row in es]
            k2t = P.tile([128, N], bf16, tag="k2t", name="k2t")
            vext = [P.tile([128, 128], bf16, tag=f"v{j}", name=f"v{j}") for j in range(JT)]
            avps = {}

            def emit_v(j):
                nc.vector.memset(vext[j][:, D:128], 0.0)
                nc.vector.memset(vext[j][:, D : D + 1], 1.0)
                ps_v = PS.tile([128, D], f32, tag="av", name="ps_v", bufs=4)
                for ft in range(FT):
                    nc.tensor.matmul(
                        ps_v,
                        lhsT=xTs(ft, slice(j * 128, (j + 1) * 128)),
                        rhs=w0s[:, ft * 2 * D + D : ft * 2 * D + 2 * D],
                        start=(ft == 0),
                        stop=(ft == FT - 1),
                    )
                nc.vector.tensor_copy(vext[j][:, 0:D], ps_v)

            def emit_av(t, j, head):
                ps_av = avps[t][head]
                nc.tensor.matmul(
                    ps_av,
                    lhsT=vext[j],
                    rhs=es_bf[t % 2][j][:, head, :],
                    start=(j == 0),
                    stop=(j == JT - 1),
                )

            def emit_pair_seg(t, j_lo, j_hi, pre=None):
                e = es[t % 2]
                e_bf = es_bf[t % 2]
                if j_lo == 0:
                    avps[t] = (
                        PS.tile([128, NQ], f32, tag="av", name="av_a", bufs=4),
                        PS.tile([128, NQ], f32, tag="av", name="av_b", bufs=4),
                    )
                for j in range(j_lo, j_hi):
                    ps_s = PS.tile([128, 2, NQ], f32, tag="s", name="s", bufs=2)
                    nc.tensor.matmul(
                        ps_s[:, 0, :],
                        lhsT=k2t[0:64, j * 128 : (j + 1) * 128],
                        rhs=qt[t][0:64, :],
                        start=True,
                        stop=True,
                    )
                    nc.tensor.matmul(
                        ps_s[:, 1, :],
                        lhsT=k2t[64:128, j * 128 : (j + 1) * 128],
                        rhs=qt[t][64:128, :],
                        start=True,
                        stop=True,
                    )
                    if dve_pick(t, j):
                        nc.vector.tensor_scalar(
                            out=e[j],
                            in0=ps_s,
                            scalar1=A_SCH,
                            scalar2=B_SCH,
                            op0=MUL,
                            op1=ADD,
                        )
                    else:
                        nc.scalar.activation(out=e_bf[j], in_=ps_s, func=EXP)
                    if pre is not None:
                        pre(j)
                    # AVs trail their exp by 2 js: the exp engines (ACT
                    # ~1113ns, DVE ~1224ns per j-tile) finish just under two
                    # PE iterations after the S matmuls, so lag 1 stalls the
                    # PE on the exp sem while lag 2 never does. Pair 0 trails
                    # deeper: its early js carry the K/c1 projection chain.
                    la, lb = (4, 5) if t == 0 else (2, 2)
                    if j >= la and (t != 0 or j - la <= 10):
                        emit_av(t, j - la, 0)
                    if j >= lb and (t != 0 or j - lb <= 9):
                        emit_av(t, j - lb, 1)
                    if 1 <= t <= 4:
                        if j == 10:
                            emit_q_part(t + 1, range(3), False)
                        if j == 12:
                            emit_q_part(t + 1, range(3, FT), True)

            def emit_spill(t_prev, j):
                # Drain pair t_prev's last AVs + output copies inside the
                # following pair's first js, keeping the PE stream dense
                # across the pair boundary (the last exps finish ~2 js after
                # their S matmuls, so these AVs can't run inside t_prev).
                if j == 0:
                    emit_av(t_prev, JT - 2, 0)
                    emit_av(t_prev, JT - 2, 1)
                if j == 1:
                    emit_av(t_prev, JT - 1, 0)
                if j == 2:
                    emit_av(t_prev, JT - 1, 1)
                if j == 4:
                    sta = W.tile([65, NQ], bf16, tag="sta", name="sta", bufs=3)
                    nc.vector.tensor_copy(sta, avps[t_prev][0][0:65, :])
                    eng_a = nc.gpsimd if t_prev % 2 == 0 else nc.sync
                    eng_a.dma_start(out=st_e[2 * t_prev][:, :], in_=sta)
                if j == 6:
                    stb = W.tile([65, NQ], bf16, tag="stb", name="stb", bufs=3)
                    nc.vector.tensor_copy(stb, avps[t_prev][1][0:65, :])
                    eng_b = nc.sync if t_prev % 2 == 0 else nc.gpsimd
                    eng_b.dma_start(out=st_e[2 * t_prev + 1][:, :], in_=stb)
                    avps.pop(t_prev)

            def emit_final_tail():
                t = H // 2 - 1
                ps_av_a, ps_av_b = avps[t]
                emit_av(t, JT - 2, 0)
                emit_av(t, JT - 2, 1)
                emit_av(t, JT - 1, 0)
                sta = W.tile([65, NQ], bf16, tag="sta", name="sta", bufs=3)
                nc.vector.tensor_copy(sta, ps_av_a[0:65, :])
                nc.sync.dma_start(out=st_e[2 * t][:, :], in_=sta)
                emit_av(t, JT - 1, 1)
                avps.pop(t)
                stb = W.tile([65, NQ], bf16, tag="stb", name="stb", bufs=3)
                # scalar engine is idle once the final exp retires: do the
                # drain copy AND the DMA there, parallel to sta's path
                nc.scalar.copy(stb, ps_av_b[0:65, :])
                nc.scalar.dma_start(out=st_e[2 * t + 1][:, :], in_=stb)

            # ---------------- K^T proj + pair 0, chunk-pipelined ----------
            # K2T[d, j]: K^T computed twice via col-tiled dual matmul groups
            # (cols 0:64 / 64:128 run concurrently) -> one [128, 512] psum.
            # V projection is emitted inside pair 0's j loop so the PE
            # stream stays dense while exps drain.
            kps = {}

            def emit_k_part(nj, fts, done, cs=None, ks=None, dup=True):
                cs = cs if cs is not None else slice(nj * 512, (nj + 1) * 512)
                if nj not in kps:
                    kps[nj] = PS.tile([128, 512], f32, tag="av", name="ps_k", bufs=4)
                ps_k = kps[nj]
                if ks is not None:
                    ps_k = ps_k[:, ks]
                for ft in fts:
                    nc.tensor.matmul(
                        ps_k[0:64, :],
                        lhsT=w0s[:, ft * 2 * D : ft * 2 * D + D],
                        rhs=xTs(ft, cs),
                        start=(ft == 0),
                        stop=(ft == FT - 1),
                        skip_group_check=True,
                    )
                    if dup:
                        nc.tensor.matmul(
                            ps_k[64:128, :],
                            lhsT=w0s[:, ft * 2 * D : ft * 2 * D + D],
                            rhs=xTs(ft, cs),
                            start=(ft == 0),
                            stop=(ft == FT - 1),
                            tile_position=(0, 64),
                            skip_group_check=True,
                        )
                if done:
                    src_ps = kps.pop(nj) if ks is None else ps_k
                    if dup:
                        nc.vector.tensor_copy(k2t[:, cs], src_ps)
                    else:
                        nc.vector.tensor_copy(k2t[0:64, cs], src_ps[0:64, :])
                        # head-b S matmuls need K on partitions 64:128 too;
                        # the gpsimd DMA queue is idle until the first output
                        nc.gpsimd.dma_start(
                            out=k2t[64:128, cs], in_=k2t[0:64, cs]
                        )

            def emit_k(nj):
                emit_k_part(nj, range(FT), True)

            # Preload the exp table set (~2.7us) during the DMA wait so the
            # first real exp doesn't pay it.
            warm_act = P.tile([128, 16], bf16, tag="warm_act", name="warm_act")
            nc.scalar.activation(out=warm_act, in_=junk[:, 0:16], func=EXP)

            # First S matmul needs qt[0] (all of chunk 0) but only keys
            # 0:128 of k2t — interleave both projections with chunk 0's
            # three-piece arrival so the PE consumes each ft pair as it
            # lands. The rest of chunk 0's keys and V follow inside pair 0's
            # early iterations.
            for p_ in range(3):
                fts = (2 * p_, 2 * p_ + 1)
                emit_k_part(0, fts, False, cs=slice(0, 128), ks=slice(0, 128))
                emit_q_part(0, fts, p_ == 2)
            nc.vector.tensor_copy(k2t[:, 0:128], kps[0][:, 0:128])

            def pair0_pre(j):
                # finish chunk 0's keys, pace V one tile per j, spread later
                # K chunks across js, and fold Q1 into the stall window
                if j == 0:
                    emit_k_part(
                        0, range(FT), False, cs=slice(128, 512), ks=slice(128, 512)
                    )
                    nc.vector.tensor_copy(k2t[:, 128:512], kps.pop(0)[:, 128:512])
                    emit_v(0)
                if j + 1 < JT:
                    emit_v(j + 1)
                # K-chunk pacing: emit each chunk's matmuls no earlier than
                # its x DMA lands (in-order PE queue — early emission blocks
                # the S stream behind a data wait). All chunks dup via the
                # dual col-tiled matmul: the second col-group is idle during
                # K projection, so the dup costs no wall time and avoids the
                # gpsimd dup-DMA latency.
                if j == 2:
                    emit_k_part(1, range(0, 3), False, dup=True)
                if j == 3:
                    emit_k_part(1, range(3, FT), True, dup=True)
                if j == 5:
                    emit_k_part(2, range(0, 3), False, dup=True)
                if j == 6:
                    emit_k_part(2, range(3, FT), True, dup=True)
                if j == 9:
                    emit_k_part(3, range(0, 3), False, dup=True)
                if j == 10:
                    emit_k_part(3, range(3, FT), True, dup=True)
                if j == 12:
                    emit_q_part(1, range(3), False)
                if j == 14:
                    emit_q_part(1, range(3, FT), True)

            emit_pair_seg(0, 0, JT, pre=pair0_pre)

            # pair 0 is PE-oversubscribed (all projections + V live there),
            # so its last AV matmuls and output drain spill into pair 1's
            # slack; the av psum pool holds both pairs' accumulators (4
            # bufs) until pair 0 drains at pair-1 j4/j5.
            def pair1_pre(j):
                if j <= 4:
                    emit_av(0, 11 + j, 0)
                if j <= 5:
                    emit_av(0, 10 + j, 1)
                if j == 4:
                    sta = W.tile([65, NQ], bf16, tag="sta", name="sta", bufs=3)
                    nc.vector.tensor_copy(sta, avps[0][0][0:65, :])
                    nc.gpsimd.dma_start(out=st_e[0][:, :], in_=sta)
                if j == 5:
                    stb = W.tile([65, NQ], bf16, tag="stb", name="stb", bufs=3)
                    nc.vector.tensor_copy(stb, avps[0][1][0:65, :])
                    nc.sync.dma_start(out=st_e[1][:, :], in_=stb)
                    avps.pop(0)

            emit_pair_seg(1, 0, JT, pre=pair1_pre)

            # ---------------- remaining pairs -----------------------------
            for t in range(2, H // 2):
                emit_pair_seg(t, 0, JT, pre=lambda j, _t=t: emit_spill(_t - 1, j))
            emit_final_tail()

    _split_multi_waits(nc)
    return nc


def make_in_maps(x, Wq, Wkv, Wproj, bproj):

    def image(a, p=128):
        # [G*p, w] -> [p, G*w] SBUF image (block g at columns g*w:(g+1)*w)
        gp, w = a.shape
        return np.ascontiguousarray(
            a.reshape(gp // p, p, w).transpose(1, 0, 2).reshape(p, -1)
        )

    wq_b = image((Wq * SCALE).astype(BF))
    # regroup to ct-major: piece ct = all six 128-row in-chunks of output
    # columns ct*128:(ct+1)*128, contiguous for piecewise DMA
    wq_b = np.ascontiguousarray(
        wq_b.reshape(128, FT, FT, 128).transpose(0, 2, 1, 3).reshape(128, FT * DIM)
    )
    wkv_b = image(Wkv.astype(BF))
    w0_b = np.ascontiguousarray(np.concatenate([wkv_b, wq_b[:, 0:DIM]], axis=1))
    wqr_b = np.ascontiguousarray(wq_b[:, DIM:])

    xTb = [x[b].T.astype(BF) for b in range(B)]

    in_maps = []
    for c in range(NCORES):
        b, q0 = c // 4, (c % 4) * NQ
        xr = np.roll(xTb[b], -q0, axis=1)  # [768, 2048]
        # image with halves outer: [128, half*6144 + ft*1024 + col]
        xi = (
            xr.reshape(FT, 128, 4, 512)
            .transpose(1, 2, 0, 3)
            .reshape(128, FT * N)
        )
        in_maps.append(
            {
                "xT": np.ascontiguousarray(xi),
                "w0": w0_b,
                "wq": wqr_b,
            }
        )
    return in_maps


def assemble_out(results, Wproj, bproj):
    Wp = Wproj.astype(np.float32)
    bp = bproj.astype(np.float32)
    out = np.empty((B, N, DIM), dtype=np.float32)
    for c in range(NCORES):
        b, q0 = c // 4, (c % 4) * NQ
        o = np.empty((NQ, DIM), dtype=np.float32)
        for h in range(H):
            st = results[c][f"st{h}"].astype(np.float32)
            o[:, h * D : (h + 1) * D] = (st[0:D] / st[D : D + 1]).T
        out[b, q0 : q0 + NQ, :] = o @ Wp + bp
    return out


def kernel(x, Wq, Wkv, Wproj, bproj, num_layer=None):
    from concourse.bass_utils import run_bass_kernel_spmd

    x = np.asarray(x, dtype=np.float32)
    Wq = np.asarray(Wq, dtype=np.float32)
    Wkv = np.asarray(Wkv, dtype=np.float32)
    Wproj = np.asarray(Wproj, dtype=np.float32)
    bproj = np.asarray(bproj, dtype=np.float32)

    in_maps = make_in_maps(x, Wq, Wkv, Wproj, bproj)
    nc = build_graph()
    res = run_bass_kernel_spmd(nc, in_maps, core_ids=list(range(NCORES)))
    return assemble_out(res.results, Wproj, bproj)



# revision 20
# speedup vs baseline: 1.0812x; 1.0247x over previous
"""MQA attention block (B=2, N=2048, DIM=768, H=12, D=64) on 8 TRN2 NeuronCores.

Sharding: batch x query-block data parallel — core c handles batch c//4,
query rows (c%4)*512..+512. Each core computes K/V for its batch locally
(redundant but cheap), all 12 heads for its query block. No collectives.

Device computes Q/K/V projections, scores, exp, and the un-normalized
AV accumulation (plus row sums via a ones column). All 12 heads' [65, 512]
(64 AV rows + 1 sums row) tiles ship to the host, which normalizes and
applies the output projection + bias (cheap: one [512,768]x[768,768] GEMM
per core).

Orientation: all tensors flow "transposed" (channels on partitions):
  QT[c,i] = Wq.T-proj, K2T[d,j] (duplicated to both partition halves),
  ST[j,i] scores -> exp on ACT -> AV via V_ext=[V|ones] giving out^T and
  row sums in one matmul.
"""

import sys

for _p in ("/opt/trn_rl_repo",):
    if _p not in sys.path:
        sys.path.insert(0, _p)

import numpy as np
import ml_dtypes

BF = ml_dtypes.bfloat16

B, N, DIM = 2, 2048, 768
H, D = 12, 64
NQ = 512            # query rows per core
SCALE = D ** -0.5
NCORES = 8
FT = DIM // 128     # 6 partition tiles of the channel dim
JT = N // 128       # 16 key tiles
NJ = N // 512       # 4


def _patch_tile_drain(tile_mod):
    """This toolchain snapshot rejects >1 sync-wait per instruction at walrus
    codegen, but TileContext's tail drain stacks every outstanding sem wait
    onto a single Drain. Split them: one drain instruction per wait."""
    import bass_rust
    from concourse.vector_clock import ScopedClock

    def _drain_and_barrier(self, tick_clock, wait_clock):
        nc = self.nc
        drain_inst = nc.sync.drain()
        wait_clock.add_sem_waits(
            drain_inst.ins, ScopedClock({None: tick_clock.global_clock})
        )
        waits = list(drain_inst.ins.sync_info.on_wait)
        if len(waits) > 1:
            drain_inst.ins.sync_info = bass_rust.SyncInfo(
                on_wait=[waits[0]], on_update=[]
            )
            for w in waits[1:]:
                extra = nc.sync.drain()
                extra.ins.sync_info = bass_rust.SyncInfo(on_wait=[w], on_update=[])
        nc.all_engine_barrier()
        assert self.sems is not None
        popped = nc._tile_sem_poison_stack.pop()
        assert popped is self._sem_poison
        nc.clear_and_free_semaphores(list(self.sems.allocated().values()))

    tile_mod.TileContext._drain_and_barrier = _drain_and_barrier


def _split_multi_waits(nc):
    """Same toolchain limitation, applied globally: walrus rejects any
    instruction carrying >1 sync-wait. Move extra waits onto fresh NoOps
    inserted just before the instruction on the same engine (engine streams
    are in-order, so this is semantically identical)."""
    from concourse import mybir

    n = 0
    for f in nc.m.functions:
        for bb in f.blocks:
            insts = bb.instructions
            out = []
            for inst in insts:
                si = inst.sync_info
                waits = list(si.on_wait) if si is not None else []
                if len(waits) > 1:
                    for w in waits[:-1]:
                        n += 1
                        out.append(
                            mybir.InstNoOp(
                                name=f"waitsplit_{n}",
                                engine=inst.engine,
                                sync_info=mybir.SyncInfo(on_wait=[w], on_update=[]),
                                bass_nofuse=True,
                            )
                        )
                    inst.sync_info = mybir.SyncInfo(
                        on_wait=[waits[-1]], on_update=list(si.on_update)
                    )
                out.append(inst)
            insts[:] = out


def build_graph():
    import concourse.bass as bass
    import concourse.tile as tile
    from concourse import mybir

    _patch_tile_drain(tile)

    f32 = mybir.dt.float32
    bf16 = mybir.dt.bfloat16
    i16 = mybir.dt.int16
    fp8 = mybir.dt.float8e4
    DR = mybir.MatmulPerfMode.DoubleRow
    EXP = mybir.ActivationFunctionType.Exp
    MUL = mybir.AluOpType.mult
    ADD = mybir.AluOpType.add

    # DVE-side softmax: Schraudolph integer exp directly into bf16 bit
    # patterns: e^s ~= bitcast_bf16(int16(round(s * 128/ln2 + (127*128 - C)))).
    # ~2% rms multiplicative error that largely cancels in the softmax
    # normalization (sums use the same approximated e). Offloads ~40% of the
    # exp stream from the saturated ACT engine onto the otherwise-idle DVE.
    A_SCH = float(2**7 / np.log(2))
    B_SCH = float(127 * 2**7 - 6.0)

    def dve_pick(t, j):
        # ACT alone paces the kernel at ~1147ns/j; handing odd js of the
        # steady pairs to the DVE (1192ns/j, off the critical ACT stream)
        # drops the softmax cadence below the PE's ~700ns/j. Pair 0's DVE is
        # loaded with k2t/vext/qt evacuations, so it only takes late js.
        if t == 0:
            return j in (9, 11, 13)
        return j % 2 == 1 and j < 15

    nc = bass.Bass()
    # all inputs arrive as exact SBUF images ([partition, free] layout built
    # on host) so each loads with one large-descriptor DMA.
    # xt image free layout: half*6144 + ft*1024 + col  (halves outer)
    xT_e = nc.declare_dram_parameter("xT", [128, FT * N], bf16, isOutput=False)
    # w0 = [wkv image | wq ct=0 piece] in one tensor: loads as a single
    # wide DMA (per-partition-line setup cost dominates small transfers)
    w0_e = nc.declare_dram_parameter("w0", [128, FT * 2 * D + DIM], bf16, isOutput=False)
    wq_e = nc.declare_dram_parameter("wq", [128, (FT - 1) * DIM], bf16, isOutput=False)
    # bf16 outputs: halves the drain-DMA bytes and doubles the cast rate;
    # ~0.4% quantization on AV+sums is far inside the error budget
    st_e = [
        nc.declare_dram_parameter(f"st{h}", [65, NQ], bf16, isOutput=True)
        for h in range(H)
    ]

    with tile.TileContext(nc) as tc:
        with (
            tc.tile_pool(name="persist", bufs=1) as P,
            tc.tile_pool(name="work", bufs=2) as W,
            tc.tile_pool(name="psum", bufs=2, space="PSUM") as PS,
        ):
            # ---------------- input loads (one DMA per tensor) -----------
            # Each logical [768, x] tensor lands as one [128, 6*x] SBUF tile
            # (f-tile ft at columns ft*x:(ft+1)*x) via a single 3D-AP DMA —
            # the ~0.6us per-dma_start sequencer issue cost dominates loads
            # otherwise. xT arrives np.roll'd per core so the query block is
            # always columns 0:NQ (softmax is key-permutation invariant).
            xt = P.tile([128, FT * N], bf16, tag="xt", name="xt")
            w0s = P.tile([128, FT * 2 * D + DIM], bf16, tag="w0s", name="w0s")
            wqs = P.tile([128, (FT - 1) * DIM], bf16, tag="wqs", name="wqs")

            def xTs(ft, sl):
                a, b_ = sl.start or 0, sl.stop
                q = a // 512
                assert (b_ - 1) // 512 == q
                base = q * 3072 + ft * 512
                return xt[:, base + a - q * 512 : base + b_ - q * 512]

            # Aggregate inbound DMA BW is ~140GB/s shared across the three
            # DMA-capable queues. Chunk 0 (queries + first keys) gates the
            # whole exp stream, so it loads as six ft-ordered pieces round-
            # robined over sync/gpsimd/scalar — the K projection consumes
            # them in arrival order. wq's ct=0 piece follows split in two;
            # later chunks ride sync/gpsimd and later wq cts ride scalar,
            # each landing just before its consumer.
            # Per-queue DMA throughput is dominated by a fixed per-
            # partition-line cost, so fewer/wider transfers win: x loads as
            # four whole chunks alternating sync/gpsimd (chunk c lands just
            # before its K-projection consumers), weights as two wide
            # transfers on the otherwise-idle scalar queue.
            # sync + scalar are hardware-DGE queues (~2x the throughput of
            # gpsimd's software DGE), so all input loads ride those two;
            # gpsimd only carries output DMAs later.
            # Arrival order tuned for the exp-stream start: w0 (wkv + wq-ct0)
            # first on scalar, chunk 0 in three ft-pair pieces on sync (the
            # K/Q projections consume pieces as they land), then the later
            # chunks paced to their first S consumers. wq ct1 rides early so
            # pair 1's Q projection never waits.
            nc.scalar.dma_start(out=w0s, in_=w0_e[:, :])
            nc.scalar.dma_start(out=wqs[:, 0:768], in_=wq_e[:, 0:768])
            nc.sync.dma_start(out=xt[:, 0:1024], in_=xT_e[:, 0:1024])
            nc.sync.dma_start(out=xt[:, 1024:2048], in_=xT_e[:, 1024:2048])
            nc.sync.dma_start(out=xt[:, 2048:3072], in_=xT_e[:, 2048:3072])
            nc.sync.dma_start(out=xt[:, 3072:4608], in_=xT_e[:, 3072:4608])
            nc.sync.dma_start(out=xt[:, 4608:6144], in_=xT_e[:, 4608:6144])
            nc.scalar.dma_start(out=xt[:, 6144:9216], in_=xT_e[:, 6144:9216])
            nc.sync.dma_start(out=xt[:, 9216:12288], in_=xT_e[:, 9216:12288])
            nc.scalar.dma_start(out=wqs[:, 768:3840], in_=wq_e[:, 768:3840])

            # ---------------- PE pre-warm -------------------
            # ~15 junk matmuls during the input-DMA wait push the PE past the
            # HAM activity window so K(0)/Q(0) run at 2.4GHz instead of 1.2.
            junk = P.tile([128, 512], bf16, tag="junk", name="junk")
            nc.vector.memset(junk, 0.5)
            warm_ps = PS.tile([128, 512], f32, tag="av", name="warm_ps", bufs=4)
            # coarse warm-up while the chunk-0 DMA is in flight, then a
            # short-matmul tail so K-A's start quantizes at ~290ns (cold
            # N=128) instead of ~630ns (cold N=512) against the arrival
            for i in range(7):
                nc.tensor.matmul(
                    warm_ps,
                    lhsT=junk[:, 0:128],
                    rhs=junk,
                    start=(i == 0),
                    stop=False,
                )
            for i in range(5):
                nc.tensor.matmul(
                    warm_ps[:, 0:128],
                    lhsT=junk[:, 0:128],
                    rhs=junk[:, 0:128],
                    start=False,
                    stop=(i == 4),
                )
            warm_out = P.tile([128, 16], f32, tag="warm_out", name="warm_out")
            nc.vector.tensor_copy(warm_out, warm_ps[:, 0:16])

            # ---------------- Q^T projection ----------------
            # qt[t] holds heads 2t (partitions 0:64) and 2t+1 (64:128).
            qt = [P.tile([128, NQ], bf16, tag=f"qt{t}", name=f"qt{t}") for t in range(FT)]

            qps = {}

            def emit_q_part(ct, fts, done):
                if ct not in qps:
                    qps[ct] = PS.tile([128, NQ], f32, tag="av", name="ps_q", bufs=4)
                ps_q = qps[ct]
                base = FT * 2 * D if ct == 0 else (ct - 1) * DIM
                wsrc = w0s if ct == 0 else wqs
                for ft in fts:
                    nc.tensor.matmul(
                        ps_q,
                        lhsT=wsrc[:, base + ft * 128 : base + (ft + 1) * 128],
                        rhs=xTs(ft, slice(0, NQ)),
                        start=(ft == 0),
                        stop=(ft == FT - 1),
                    )
                if done:
                    # steady-pair qt evacuations ride the scalar engine's
                    # slack: the DVE is near-saturated with Schraudolph exps,
                    # and a copy queued behind them stalls the S stream via
                    # the ps_s WAR chain. Pair 0/1's copies stay on the DVE
                    # (its ACT is the tighter engine there).
                    eng = nc.vector if ct <= 1 else nc.scalar
                    if eng is nc.vector:
                        eng.tensor_copy(qt[ct], qps.pop(ct))
                    else:
                        eng.copy(qt[ct], qps.pop(ct))

            def emit_q(ct):
                emit_q_part(ct, range(FT), True)

            # ---------------- attention emitters --------------------------
            # Per pair t: heads a=2t (partitions 0:64 of qt[t]) and b=2t+1
            # (64:128). Per j: two S matmuls (row groups 0/64) into one
            # [128, 2, 512] psum tile, one exp for both; AV matmuls
            # (lhsT=[V|ones] -> psum rows 0:64 out^T + row 64 sums) trail
            # the exps by 1 (head a) / 2 (head b). AV psums stage to SBUF
            # at the pair tail and DMA to DRAM; host normalizes + projects.
            # e tiles are allocated int16 with a bf16 bitcast view over the
            # same bytes: ACT writes exp() through the bf16 view, the DVE
            # writes Schraudolph int16 bit patterns natively, and the AV
            # matmuls always read the bf16 view.
            es = [
                [W.tile([128, 2, NQ], i16, tag=f"e{j}", name=f"e{j}", bufs=2) for j in range(JT)]
                for _ in range(2)
            ]
            es_bf = [[e.bitcast(bf16) for e in row] for row in es]
            k2t = P.tile([128, N], bf16, tag="k2t", name="k2t")
            vext = [P.tile([128, 128], bf16, tag=f"v{j}", name=f"v{j}") for j in range(JT)]
            avps = {}

            def emit_v(j):
                nc.vector.memset(vext[j][:, D:128], 0.0)
                nc.vector.memset(vext[j][:, D : D + 1], 1.0)
                ps_v = PS.tile([128, D], f32, tag="av", name="ps_v", bufs=4)
                for ft in range(FT):
                    nc.tensor.matmul(
                        ps_v,
                        lhsT=xTs(ft, slice(j * 128, (j + 1) * 128)),
                        rhs=w0s[:, ft * 2 * D + D : ft * 2 * D + 2 * D],
                        start=(ft == 0),
                        stop=(ft == FT - 1),
                    )
                nc.vector.tensor_copy(vext[j][:, 0:D], ps_v)

            def emit_av(t, j, head):
                ps_av = avps[t][head]
                nc.tensor.matmul(
                    ps_av,
                    lhsT=vext[j],
                    rhs=es_bf[t % 2][j][:, head, :],
                    start=(j == 0),
                    stop=(j == JT - 1),
                )

            def emit_pair_seg(t, j_lo, j_hi, pre=None):
                e = es[t % 2]
                e_bf = es_bf[t % 2]
                if j_lo == 0:
                    avps[t] = (
                        PS.tile([128, NQ], f32, tag="av", name="av_a", bufs=4),
                        PS.tile([128, NQ], f32, tag="av", name="av_b", bufs=4),
                    )
                # js are processed in groups of two, batching the PE stream
                # by array mode: both js' S-pairs (64-row mode) issue
                # back-to-back, then all full-128 work (AVs, projections).
                # Each 64<->128 mode change drains the array (~140ns), so
                # halving the transitions saves ~70ns/j; the row-tiled S
                # matmuls of one j stream concurrently (~213ns/pair).
                assert j_lo % 2 == 0 and j_hi % 2 == 0
                for g in range(j_lo // 2, j_hi // 2):
                    group = (2 * g, 2 * g + 1)
                    pss = {}
                    for j in group:
                        ps_s = PS.tile([128, 2, NQ], f32, tag="s", name="s", bufs=2)
                        pss[j] = ps_s
                        nc.tensor.matmul(
                            ps_s[:, 0, :],
                            lhsT=k2t[0:64, j * 128 : (j + 1) * 128],
                            rhs=qt[t][0:64, :],
                            start=True,
                            stop=True,
                        )
                        nc.tensor.matmul(
                            ps_s[:, 1, :],
                            lhsT=k2t[64:128, j * 128 : (j + 1) * 128],
                            rhs=qt[t][64:128, :],
                            start=True,
                            stop=True,
                        )
                    for j in group:
                        if dve_pick(t, j):
                            nc.vector.tensor_scalar(
                                out=e[j],
                                in0=pss[j],
                                scalar1=A_SCH,
                                scalar2=B_SCH,
                                op0=MUL,
                                op1=ADD,
                            )
                        else:
                            nc.scalar.activation(out=e_bf[j], in_=pss[j], func=EXP)
                    for j in group:
                        if pre is not None:
                            pre(j)
                        # AVs trail their exp by 2 js: the exp engines (ACT
                        # ~1113ns, DVE ~1224ns per j-tile) finish just under
                        # two PE iterations after the S matmuls, so lag 1
                        # stalls the PE on the exp sem while lag 2 never
                        # does. Pair 0 trails deeper: its early js carry the
                        # K/c1 projection chain.
                        la, lb = (4, 5) if t == 0 else (2, 2)
                        if j >= la and (t != 0 or j - la <= 10):
                            emit_av(t, j - la, 0)
                        if j >= lb and (t != 0 or j - lb <= 9):
                            emit_av(t, j - lb, 1)
                        if 1 <= t <= 4:
                            if j == 10:
                                emit_q_part(t + 1, range(3), False)
                            if j == 12:
                                emit_q_part(t + 1, range(3, FT), True)

            def emit_spill(t_prev, j):
                # Drain pair t_prev's last AVs + output copies inside the
                # following pair's first js, keeping the PE stream dense
                # across the pair boundary (the last exps finish ~2 js after
                # their S matmuls, so these AVs can't run inside t_prev).
                if j == 0:
                    emit_av(t_prev, JT - 2, 0)
                    emit_av(t_prev, JT - 2, 1)
                if j == 1:
                    emit_av(t_prev, JT - 1, 0)
                if j == 2:
                    emit_av(t_prev, JT - 1, 1)
                if j == 4:
                    # sta evac on scalar: ACT has ~2us/pair of slack, the
                    # DVE (Schraudolph exps) does not.
                    sta = W.tile([65, NQ], bf16, tag="sta", name="sta", bufs=3)
                    nc.scalar.copy(sta, avps[t_prev][0][0:65, :])
                    eng_a = nc.gpsimd if t_prev % 2 == 0 else nc.sync
                    eng_a.dma_start(out=st_e[2 * t_prev][:, :], in_=sta)
                if j == 6:
                    stb = W.tile([65, NQ], bf16, tag="stb", name="stb", bufs=3)
                    nc.vector.tensor_copy(stb, avps[t_prev][1][0:65, :])
                    eng_b = nc.sync if t_prev % 2 == 0 else nc.gpsimd
                    eng_b.dma_start(out=st_e[2 * t_prev + 1][:, :], in_=stb)
                    avps.pop(t_prev)

            def emit_final_tail():
                t = H // 2 - 1
                ps_av_a, ps_av_b = avps[t]
                emit_av(t, JT - 2, 0)
                emit_av(t, JT - 2, 1)
                emit_av(t, JT - 1, 0)
                sta = W.tile([65, NQ], bf16, tag="sta", name="sta", bufs=3)
                nc.vector.tensor_copy(sta, ps_av_a[0:65, :])
                nc.sync.dma_start(out=st_e[2 * t][:, :], in_=sta)
                emit_av(t, JT - 1, 1)
                avps.pop(t)
                stb = W.tile([65, NQ], bf16, tag="stb", name="stb", bufs=3)
                # scalar engine is idle once the final exp retires: do the
                # drain copy AND the DMA there, parallel to sta's path
                nc.scalar.copy(stb, ps_av_b[0:65, :])
                nc.scalar.dma_start(out=st_e[2 * t + 1][:, :], in_=stb)

            # ---------------- K^T proj + pair 0, chunk-pipelined ----------
            # K2T[d, j]: K^T computed twice via col-tiled dual matmul groups
            # (cols 0:64 / 64:128 run concurrently) -> one [128, 512] psum.
            # V projection is emitted inside pair 0's j loop so the PE
            # stream stays dense while exps drain.
            kps = {}

            def emit_k_part(nj, fts, done, cs=None, ks=None, dup=True):
                cs = cs if cs is not None else slice(nj * 512, (nj + 1) * 512)
                if nj not in kps:
                    kps[nj] = PS.tile([128, 512], f32, tag="av", name="ps_k", bufs=4)
                ps_k = kps[nj]
                if ks is not None:
                    ps_k = ps_k[:, ks]
                for ft in fts:
                    nc.tensor.matmul(
                        ps_k[0:64, :],
                        lhsT=w0s[:, ft * 2 * D : ft * 2 * D + D],
                        rhs=xTs(ft, cs),
                        start=(ft == 0),
                        stop=(ft == FT - 1),
                        skip_group_check=True,
                    )
                    if dup:
                        nc.tensor.matmul(
                            ps_k[64:128, :],
                            lhsT=w0s[:, ft * 2 * D : ft * 2 * D + D],
                            rhs=xTs(ft, cs),
                            start=(ft == 0),
                            stop=(ft == FT - 1),
                            tile_position=(0, 64),
                            skip_group_check=True,
                        )
                if done:
                    src_ps = kps.pop(nj) if ks is None else ps_k
                    if dup:
                        nc.vector.tensor_copy(k2t[:, cs], src_ps)
                    else:
                        nc.vector.tensor_copy(k2t[0:64, cs], src_ps[0:64, :])
                        # head-b S matmuls need K on partitions 64:128 too;
                        # the gpsimd DMA queue is idle until the first output
                        nc.gpsimd.dma_start(
                            out=k2t[64:128, cs], in_=k2t[0:64, cs]
                        )

            def emit_k(nj):
                emit_k_part(nj, range(FT), True)

            # Preload the exp table set (~2.7us) during the DMA wait so the
            # first real exp doesn't pay it.
            warm_act = P.tile([128, 16], bf16, tag="warm_act", name="warm_act")
            nc.scalar.activation(out=warm_act, in_=junk[:, 0:16], func=EXP)

            # First S matmul needs qt[0] (all of chunk 0) but only keys
            # 0:128 of k2t — interleave both projections with chunk 0's
            # three-piece arrival so the PE consumes each ft pair as it
            # lands. The rest of chunk 0's keys and V follow inside pair 0's
            # early iterations.
            for p_ in range(3):
                fts = (2 * p_, 2 * p_ + 1)
                emit_k_part(0, fts, False, cs=slice(0, 256), ks=slice(0, 256))
                emit_q_part(0, fts, p_ == 2)
            nc.vector.tensor_copy(k2t[:, 0:256], kps[0][:, 0:256])

            def pair0_pre(j):
                # finish chunk 0's keys, pace V one tile per j, spread later
                # K chunks across js, and fold Q1 into the stall window
                if j == 0:
                    emit_k_part(
                        0, range(FT), False, cs=slice(256, 512), ks=slice(256, 512)
                    )
                    nc.vector.tensor_copy(k2t[:, 256:512], kps.pop(0)[:, 256:512])
                    emit_v(0)
                if j + 1 < JT:
                    emit_v(j + 1)
                # K-chunk pacing: emit each chunk's matmuls no earlier than
                # its x DMA lands (in-order PE queue — early emission blocks
                # the S stream behind a data wait). All chunks dup via the
                # dual col-tiled matmul: the second col-group is idle during
                # K projection, so the dup costs no wall time and avoids the
                # gpsimd dup-DMA latency.
                if j == 2:
                    emit_k_part(1, range(0, 3), False, dup=True)
                if j == 3:
                    emit_k_part(1, range(3, FT), True, dup=True)
                if j == 5:
                    emit_k_part(2, range(0, 3), False, dup=True)
                if j == 6:
                    emit_k_part(2, range(3, FT), True, dup=True)
                if j == 9:
                    emit_k_part(3, range(0, 3), False, dup=True)
                if j == 10:
                    emit_k_part(3, range(3, FT), True, dup=True)
                if j == 12:
                    emit_q_part(1, range(3), False)
                if j == 14:
                    emit_q_part(1, range(3, FT), True)

            emit_pair_seg(0, 0, JT, pre=pair0_pre)

            # pair 0 is PE-oversubscribed (all projections + V live there),
            # so its last AV matmuls and output drain spill into pair 1's
            # slack; the av psum pool holds both pairs' accumulators (4
            # bufs) until pair 0 drains at pair-1 j4/j5.
            def pair1_pre(j):
                if j <= 4:
                    emit_av(0, 11 + j, 0)
                if j <= 5:
                    emit_av(0, 10 + j, 1)
                if j == 4:
                    sta = W.tile([65, NQ], bf16, tag="sta", name="sta", bufs=3)
                    nc.scalar.copy(sta, avps[0][0][0:65, :])
                    nc.gpsimd.dma_start(out=st_e[0][:, :], in_=sta)
                if j == 5:
                    stb = W.tile([65, NQ], bf16, tag="stb", name="stb", bufs=3)
                    nc.vector.tensor_copy(stb, avps[0][1][0:65, :])
                    nc.sync.dma_start(out=st_e[1][:, :], in_=stb)
                    avps.pop(0)

            emit_pair_seg(1, 0, JT, pre=pair1_pre)

            # ---------------- remaining pairs -----------------------------
            for t in range(2, H // 2):
                emit_pair_seg(t, 0, JT, pre=lambda j, _t=t: emit_spill(_t - 1, j))
            emit_final_tail()

    _split_multi_waits(nc)
    return nc


def make_in_maps(x, Wq, Wkv, Wproj, bproj):

    def image(a, p=128):
        # [G*p, w] -> [p, G*w] SBUF image (block g at columns g*w:(g+1)*w)
        gp, w = a.shape
        return np.ascontiguousarray(
            a.reshape(gp // p, p, w).transpose(1, 0, 2).reshape(p, -1)
        )

    wq_b = image((Wq * SCALE).astype(BF))
    # regroup to ct-major: piece ct = all six 128-row in-chunks of output
    # columns ct*128:(ct+1)*128, contiguous for piecewise DMA
    wq_b = np.ascontiguousarray(
        wq_b.reshape(128, FT, FT, 128).transpose(0, 2, 1, 3).reshape(128, FT * DIM)
    )
    wkv_b = image(Wkv.astype(BF))
    w0_b = np.ascontiguousarray(np.concatenate([wkv_b, wq_b[:, 0:DIM]], axis=1))
    wqr_b = np.ascontiguousarray(wq_b[:, DIM:])

    xTb = [x[b].T.astype(BF) for b in range(B)]

    in_maps = []
    for c in range(NCORES):
        b, q0 = c // 4, (c % 4) * NQ
        xr = np.roll(xTb[b], -q0, axis=1)  # [768, 2048]
        # image with halves outer: [128, half*6144 + ft*1024 + col]
        xi = (
            xr.reshape(FT, 128, 4, 512)
            .transpose(1, 2, 0, 3)
            .reshape(128, FT * N)
        )
        in_maps.append(
            {
                "xT": np.ascontiguousarray(xi),
                "w0": w0_b,
                "wq": wqr_b,
            }
        )
    return in_maps


def assemble_out(results, Wproj, bproj):
    Wp = Wproj.astype(np.float32)
    bp = bproj.astype(np.float32)
    out = np.empty((B, N, DIM), dtype=np.float32)
    for c in range(NCORES):
        b, q0 = c // 4, (c % 4) * NQ
        o = np.empty((NQ, DIM), dtype=np.float32)
        for h in range(H):
            st = results[c][f"st{h}"].astype(np.float32)
            o[:, h * D : (h + 1) * D] = (st[0:D] / st[D : D + 1]).T
        out[b, q0 : q0 + NQ, :] = o @ Wp + bp
    return out


def kernel(x, Wq, Wkv, Wproj, bproj, num_layer=None):
    from concourse.bass_utils import run_bass_kernel_spmd

    x = np.asarray(x, dtype=np.float32)
    Wq = np.asarray(Wq, dtype=np.float32)
    Wkv = np.asarray(Wkv, dtype=np.float32)
    Wproj = np.asarray(Wproj, dtype=np.float32)
    bproj = np.asarray(bproj, dtype=np.float32)

    in_maps = make_in_maps(x, Wq, Wkv, Wproj, bproj)
    nc = build_graph()
    res = run_bass_kernel_spmd(nc, in_maps, core_ids=list(range(NCORES)))
    return assemble_out(res.results, Wproj, bproj)



# revision 24
# speedup vs baseline: 1.1526x; 1.0661x over previous
"""MQA attention block (B=2, N=2048, DIM=768, H=12, D=64) on 8 TRN2 NeuronCores.

Sharding: batch x query-block data parallel — core c handles batch c//4,
query rows (c%4)*512..+512. Each core computes K/V for its batch locally
(redundant but cheap), all 12 heads for its query block. No collectives.

Device computes Q/K/V projections, scores, exp, and the un-normalized
AV accumulation (plus row sums via a ones column). All 12 heads' [65, 512]
(64 AV rows + 1 sums row) tiles ship to the host, which normalizes and
applies the output projection + bias (cheap: one [512,768]x[768,768] GEMM
per core).

Orientation: all tensors flow "transposed" (channels on partitions):
  QT[c,i] = Wq.T-proj, K2T[d,j] (duplicated to both partition halves),
  ST[j,i] scores -> exp on ACT -> AV via V_ext=[V|ones] giving out^T and
  row sums in one matmul.
"""

import sys

for _p in ("/opt/trn_rl_repo",):
    if _p not in sys.path:
        sys.path.insert(0, _p)

import numpy as np
import ml_dtypes

BF = ml_dtypes.bfloat16

B, N, DIM = 2, 2048, 768
H, D = 12, 64
NQ = 512            # query rows per core
SCALE = D ** -0.5
NCORES = 8
FT = DIM // 128     # 6 partition tiles of the channel dim
JT = N // 128       # 16 key tiles
NJ = N // 512       # 4


def _patch_tile_drain(tile_mod):
    """This toolchain snapshot rejects >1 sync-wait per instruction at walrus
    codegen, but TileContext's tail drain stacks every outstanding sem wait
    onto a single Drain. Split them: one drain instruction per wait."""
    import bass_rust
    from concourse.vector_clock import ScopedClock

    def _drain_and_barrier(self, tick_clock, wait_clock):
        nc = self.nc
        drain_inst = nc.sync.drain()
        wait_clock.add_sem_waits(
            drain_inst.ins, ScopedClock({None: tick_clock.global_clock})
        )
        waits = list(drain_inst.ins.sync_info.on_wait)
        if len(waits) > 1:
            drain_inst.ins.sync_info = bass_rust.SyncInfo(
                on_wait=[waits[0]], on_update=[]
            )
            for w in waits[1:]:
                extra = nc.sync.drain()
                extra.ins.sync_info = bass_rust.SyncInfo(on_wait=[w], on_update=[])
        nc.all_engine_barrier()
        assert self.sems is not None
        popped = nc._tile_sem_poison_stack.pop()
        assert popped is self._sem_poison
        nc.clear_and_free_semaphores(list(self.sems.allocated().values()))

    tile_mod.TileContext._drain_and_barrier = _drain_and_barrier


def _split_multi_waits(nc):
    """Same toolchain limitation, applied globally: walrus rejects any
    instruction carrying >1 sync-wait. Move extra waits onto fresh NoOps
    inserted just before the instruction on the same engine (engine streams
    are in-order, so this is semantically identical)."""
    from concourse import mybir

    n = 0
    for f in nc.m.functions:
        for bb in f.blocks:
            insts = bb.instructions
            out = []
            for inst in insts:
                si = inst.sync_info
                waits = list(si.on_wait) if si is not None else []
                if len(waits) > 1:
                    for w in waits[:-1]:
                        n += 1
                        out.append(
                            mybir.InstNoOp(
                                name=f"waitsplit_{n}",
                                engine=inst.engine,
                                sync_info=mybir.SyncInfo(on_wait=[w], on_update=[]),
                                bass_nofuse=True,
                            )
                        )
                    inst.sync_info = mybir.SyncInfo(
                        on_wait=[waits[-1]], on_update=list(si.on_update)
                    )
                out.append(inst)
            insts[:] = out


def build_graph():
    import concourse.bass as bass
    import concourse.tile as tile
    from concourse import mybir

    _patch_tile_drain(tile)

    f32 = mybir.dt.float32
    bf16 = mybir.dt.bfloat16
    i16 = mybir.dt.int16
    fp8 = mybir.dt.float8e4
    DR = mybir.MatmulPerfMode.DoubleRow
    EXP = mybir.ActivationFunctionType.Exp
    MUL = mybir.AluOpType.mult
    ADD = mybir.AluOpType.add

    # DVE-side softmax: Schraudolph integer exp directly into bf16 bit
    # patterns: e^s ~= bitcast_bf16(int16(round(s * 128/ln2 + (127*128 - C)))).
    # ~2% rms multiplicative error that largely cancels in the softmax
    # normalization (sums use the same approximated e). Offloads ~half the
    # exp stream from the saturated ACT engine onto the otherwise-idle DVE.
    #
    # The split is per HEAD, not per j: head a's scores go in their own
    # single-bank psum tile read by ACT (exact exp), head b's in another
    # read by the DVE. Per-head FD=512 exps halve the exp latency on the
    # ps_s WAR chain (S(j+2) reuses exp(j)'s bank), which would otherwise
    # pace the whole kernel at ~968ns/j.
    A_SCH = float(2**7 / np.log(2))
    B_SCH = float(127 * 2**7 - 6.0)

    def dve_pick(t, j):
        # head b's exp engine: DVE everywhere except a few pair-0 js, where
        # the DVE is loaded with k2t/vext/qt evacuations and ACT (which only
        # carries head-a exps now) has slack against the DMA-paced cadence.
        if t == 0:
            return j not in (2, 4, 6, 8, 10, 12)
        return True

    nc = bass.Bass()
    # all inputs arrive as exact SBUF images ([partition, free] layout built
    # on host) so each loads with one large-descriptor DMA.
    # xt image free layout: half*6144 + ft*1024 + col  (halves outer)
    xT_e = nc.declare_dram_parameter("xT", [128, FT * N], bf16, isOutput=False)
    # w0 = [wkv image | wq ct=0 piece] in one tensor: loads as a single
    # wide DMA (per-partition-line setup cost dominates small transfers)
    w0_e = nc.declare_dram_parameter("w0", [128, FT * 2 * D + DIM], bf16, isOutput=False)
    wq_e = nc.declare_dram_parameter("wq", [128, (FT - 1) * DIM], bf16, isOutput=False)
    # bf16 outputs: halves the drain-DMA bytes and doubles the cast rate;
    # ~0.4% quantization on AV+sums is far inside the error budget
    st_e = [
        nc.declare_dram_parameter(f"st{h}", [65, NQ], bf16, isOutput=True)
        for h in range(H)
    ]

    with tile.TileContext(nc) as tc:
        with (
            tc.tile_pool(name="persist", bufs=1) as P,
            tc.tile_pool(name="work", bufs=2) as W,
            tc.tile_pool(name="psum", bufs=2, space="PSUM") as PS,
        ):
            # ---------------- input loads (one DMA per tensor) -----------
            # Each logical [768, x] tensor lands as one [128, 6*x] SBUF tile
            # (f-tile ft at columns ft*x:(ft+1)*x) via a single 3D-AP DMA —
            # the ~0.6us per-dma_start sequencer issue cost dominates loads
            # otherwise. xT arrives np.roll'd per core so the query block is
            # always columns 0:NQ (softmax is key-permutation invariant).
            xt = P.tile([128, FT * N], bf16, tag="xt", name="xt")
            w0s = P.tile([128, FT * 2 * D + DIM], bf16, tag="w0s", name="w0s")
            wqs = P.tile([128, (FT - 1) * DIM], bf16, tag="wqs", name="wqs")

            def xTs(ft, sl):
                a, b_ = sl.start or 0, sl.stop
                q = a // 512
                assert (b_ - 1) // 512 == q
                base = q * 3072 + ft * 512
                return xt[:, base + a - q * 512 : base + b_ - q * 512]

            # Aggregate inbound DMA BW is ~140GB/s shared across the three
            # DMA-capable queues. Chunk 0 (queries + first keys) gates the
            # whole exp stream, so it loads as six ft-ordered pieces round-
            # robined over sync/gpsimd/scalar — the K projection consumes
            # them in arrival order. wq's ct=0 piece follows split in two;
            # later chunks ride sync/gpsimd and later wq cts ride scalar,
            # each landing just before its consumer.
            # Per-queue DMA throughput is dominated by a fixed per-
            # partition-line cost, so fewer/wider transfers win: x loads as
            # four whole chunks alternating sync/gpsimd (chunk c lands just
            # before its K-projection consumers), weights as two wide
            # transfers on the otherwise-idle scalar queue.
            # sync + scalar are hardware-DGE queues (~2x the throughput of
            # gpsimd's software DGE), so all input loads ride those two;
            # gpsimd only carries output DMAs later.
            # Arrival order tuned for the exp-stream start: w0 (wkv + wq-ct0)
            # first on scalar, chunk 0 in three ft-pair pieces on sync (the
            # K/Q projections consume pieces as they land), then the later
            # chunks paced to their first S consumers. wq ct1 rides early so
            # pair 1's Q projection never waits.
            nc.scalar.dma_start(out=w0s, in_=w0_e[:, :])
            nc.scalar.dma_start(out=wqs[:, 0:768], in_=wq_e[:, 0:768])
            nc.sync.dma_start(out=xt[:, 0:1024], in_=xT_e[:, 0:1024])
            nc.sync.dma_start(out=xt[:, 1024:2048], in_=xT_e[:, 1024:2048])
            nc.sync.dma_start(out=xt[:, 2048:3072], in_=xT_e[:, 2048:3072])
            nc.sync.dma_start(out=xt[:, 3072:4608], in_=xT_e[:, 3072:4608])
            nc.sync.dma_start(out=xt[:, 4608:6144], in_=xT_e[:, 4608:6144])
            nc.scalar.dma_start(out=xt[:, 6144:9216], in_=xT_e[:, 6144:9216])
            nc.sync.dma_start(out=xt[:, 9216:12288], in_=xT_e[:, 9216:12288])
            nc.scalar.dma_start(out=wqs[:, 768:3840], in_=wq_e[:, 768:3840])

            # ---------------- PE pre-warm -------------------
            # ~15 junk matmuls during the input-DMA wait push the PE past the
            # HAM activity window so K(0)/Q(0) run at 2.4GHz instead of 1.2.
            junk = P.tile([128, 512], bf16, tag="junk", name="junk")
            nc.vector.memset(junk, 0.5)
            warm_ps = PS.tile([128, 512], f32, tag="av", name="warm_ps", bufs=4)
            # coarse warm-up while the chunk-0 DMA is in flight, then a
            # short-matmul tail so K-A's start quantizes at ~290ns (cold
            # N=128) instead of ~630ns (cold N=512) against the arrival
            # 10 long + 5 short junk matmuls: spans ~8.4-13.6us, bridging the
            # idle gap to the ~13-14us x/w0 arrival so the HAM never
            # re-throttles and the K/Q projections run at 2.4GHz (measured:
            # with the shorter warm-up they ran cold 15.5-22.4us at 1.2GHz).
            for i in range(10):
                nc.tensor.matmul(
                    warm_ps,
                    lhsT=junk[:, 0:128],
                    rhs=junk,
                    start=(i == 0),
                    stop=False,
                )
            for i in range(5):
                nc.tensor.matmul(
                    warm_ps[:, 0:128],
                    lhsT=junk[:, 0:128],
                    rhs=junk[:, 0:128],
                    start=False,
                    stop=(i == 4),
                )
            warm_out = P.tile([128, 16], f32, tag="warm_out", name="warm_out")
            nc.vector.tensor_copy(warm_out, warm_ps[:, 0:16])

            # ---------------- Q^T projection ----------------
            # qt[t] holds heads 2t (partitions 0:64) and 2t+1 (64:128).
            qt = [P.tile([128, NQ], bf16, tag=f"qt{t}", name=f"qt{t}") for t in range(FT)]

            qps = {}

            def emit_q_part(ct, fts, done):
                if ct not in qps:
                    qps[ct] = PS.tile([128, NQ], f32, tag="av", name="ps_q", bufs=4)
                ps_q = qps[ct]
                base = FT * 2 * D if ct == 0 else (ct - 1) * DIM
                wsrc = w0s if ct == 0 else wqs
                for ft in fts:
                    nc.tensor.matmul(
                        ps_q,
                        lhsT=wsrc[:, base + ft * 128 : base + (ft + 1) * 128],
                        rhs=xTs(ft, slice(0, NQ)),
                        start=(ft == 0),
                        stop=(ft == FT - 1),
                    )
                if done:
                    nc.vector.tensor_copy(qt[ct], qps.pop(ct))

            def emit_q(ct):
                emit_q_part(ct, range(FT), True)

            # ---------------- attention emitters --------------------------
            # Per pair t: heads a=2t (partitions 0:64 of qt[t]) and b=2t+1
            # (64:128). Per j: two S matmuls (row groups 0/64) into one
            # [128, 2, 512] psum tile, one exp for both; AV matmuls
            # (lhsT=[V|ones] -> psum rows 0:64 out^T + row 64 sums) trail
            # the exps by 1 (head a) / 2 (head b). AV psums stage to SBUF
            # at the pair tail and DMA to DRAM; host normalizes + projects.
            # e tiles are allocated int16 with a bf16 bitcast view over the
            # same bytes: ACT writes exp() through the bf16 view, the DVE
            # writes Schraudolph int16 bit patterns natively, and the AV
            # matmuls always read the bf16 view.
            es = [
                [W.tile([128, 2, NQ], i16, tag=f"e{j}", name=f"e{j}", bufs=2) for j in range(JT)]
                for _ in range(2)
            ]
            es_bf = [[e.bitcast(bf16) for e in row] for row in es]
            k2t = P.tile([128, N], bf16, tag="k2t", name="k2t")
            vext = [P.tile([128, 128], bf16, tag=f"v{j}", name=f"v{j}") for j in range(JT)]
            avps = {}

            def emit_v(j):
                nc.vector.memset(vext[j][:, D:128], 0.0)
                nc.vector.memset(vext[j][:, D : D + 1], 1.0)
                ps_v = PS.tile([128, D], f32, tag="av", name="ps_v", bufs=4)
                for ft in range(FT):
                    nc.tensor.matmul(
                        ps_v,
                        lhsT=xTs(ft, slice(j * 128, (j + 1) * 128)),
                        rhs=w0s[:, ft * 2 * D + D : ft * 2 * D + 2 * D],
                        start=(ft == 0),
                        stop=(ft == FT - 1),
                    )
                nc.vector.tensor_copy(vext[j][:, 0:D], ps_v)

            def emit_av(t, j, head):
                ps_av = avps[t][head]
                nc.tensor.matmul(
                    ps_av,
                    lhsT=vext[j],
                    rhs=es_bf[t % 2][j][:, head, :],
                    start=(j == 0),
                    stop=(j == JT - 1),
                )

            def emit_pair_seg(t, j_lo, j_hi, pre=None):
                e = es[t % 2]
                e_bf = es_bf[t % 2]
                if j_lo == 0:
                    avps[t] = (
                        PS.tile([128, NQ], f32, tag="av", name="av_a", bufs=4),
                        PS.tile([128, NQ], f32, tag="av", name="av_b", bufs=4),
                    )
                # js are processed in groups of two, batching the PE stream
                # by array mode: both js' S-pairs (64-row mode) issue
                # back-to-back, then all full-128 work (AVs, projections).
                # Each 64<->128 mode change drains the array (~140ns), so
                # halving the transitions saves ~70ns/j; the row-tiled S
                # matmuls of one j stream concurrently (~213ns/pair).
                assert j_lo % 2 == 0 and j_hi % 2 == 0
                for g in range(j_lo // 2, j_hi // 2):
                    group = (2 * g, 2 * g + 1)
                    pss = {}
                    for j in group:
                        ps_a = PS.tile([128, NQ], f32, tag="sa", name="sa", bufs=2)
                        ps_b = PS.tile([128, NQ], f32, tag="sb", name="sb", bufs=2)
                        pss[j] = (ps_a, ps_b)
                        nc.tensor.matmul(
                            ps_a,
                            lhsT=k2t[0:64, j * 128 : (j + 1) * 128],
                            rhs=qt[t][0:64, :],
                            start=True,
                            stop=True,
                        )
                        nc.tensor.matmul(
                            ps_b,
                            lhsT=k2t[64:128, j * 128 : (j + 1) * 128],
                            rhs=qt[t][64:128, :],
                            start=True,
                            stop=True,
                        )
                    for j in group:
                        # head a: exact exp on ACT; head b: Schraudolph on
                        # the DVE (or ACT for a few evacuation-heavy pair-0
                        # js). Both engines run concurrently every j.
                        nc.scalar.activation(
                            out=e_bf[j][:, 0, :], in_=pss[j][0], func=EXP
                        )
                        if dve_pick(t, j):
                            nc.vector.tensor_scalar(
                                out=e[j][:, 1, :],
                                in0=pss[j][1],
                                scalar1=A_SCH,
                                scalar2=B_SCH,
                                op0=MUL,
                                op1=ADD,
                            )
                        else:
                            nc.scalar.activation(
                                out=e_bf[j][:, 1, :], in_=pss[j][1], func=EXP
                            )
                    for j in group:
                        if pre is not None:
                            pre(j)
                        # AVs trail their exp by 2 js: the exp engines (ACT
                        # ~1113ns, DVE ~1224ns per j-tile) finish just under
                        # two PE iterations after the S matmuls, so lag 1
                        # stalls the PE on the exp sem while lag 2 never
                        # does. Pair 0 trails deeper: its early js carry the
                        # K/c1 projection chain.
                        la, lb = (4, 5) if t == 0 else (2, 2)
                        if j >= la and (t != 0 or j - la <= 10):
                            emit_av(t, j - la, 0)
                        if j >= lb and (t != 0 or j - lb <= 9):
                            emit_av(t, j - lb, 1)
                        if 1 <= t <= 4:
                            if j == 10:
                                emit_q_part(t + 1, range(3), False)
                            if j == 12:
                                emit_q_part(t + 1, range(3, FT), True)

            def emit_spill(t_prev, j):
                # Drain pair t_prev's last AVs + output copies inside the
                # following pair's first js, keeping the PE stream dense
                # across the pair boundary (the last exps finish ~2 js after
                # their S matmuls, so these AVs can't run inside t_prev).
                if j == 0:
                    emit_av(t_prev, JT - 2, 0)
                    emit_av(t_prev, JT - 2, 1)
                if j == 1:
                    emit_av(t_prev, JT - 1, 0)
                if j == 2:
                    emit_av(t_prev, JT - 1, 1)
                if j == 4:
                    # sta evac on scalar: ACT has ~2us/pair of slack, the
                    # DVE (Schraudolph exps) does not.
                    sta = W.tile([65, NQ], bf16, tag="sta", name="sta", bufs=3)
                    nc.scalar.copy(sta, avps[t_prev][0][0:65, :])
                    eng_a = nc.gpsimd if t_prev % 2 == 0 else nc.sync
                    eng_a.dma_start(out=st_e[2 * t_prev][:, :], in_=sta)
                if j == 6:
                    stb = W.tile([65, NQ], bf16, tag="stb", name="stb", bufs=3)
                    nc.vector.tensor_copy(stb, avps[t_prev][1][0:65, :])
                    eng_b = nc.sync if t_prev % 2 == 0 else nc.gpsimd
                    eng_b.dma_start(out=st_e[2 * t_prev + 1][:, :], in_=stb)
                    avps.pop(t_prev)

            def emit_final_tail():
                t = H // 2 - 1
                ps_av_a, ps_av_b = avps[t]
                emit_av(t, JT - 2, 0)
                emit_av(t, JT - 2, 1)
                emit_av(t, JT - 1, 0)
                sta = W.tile([65, NQ], bf16, tag="sta", name="sta", bufs=3)
                nc.vector.tensor_copy(sta, ps_av_a[0:65, :])
                nc.sync.dma_start(out=st_e[2 * t][:, :], in_=sta)
                emit_av(t, JT - 1, 1)
                avps.pop(t)
                stb = W.tile([65, NQ], bf16, tag="stb", name="stb", bufs=3)
                # scalar engine is idle once the final exp retires: do the
                # drain copy AND the DMA there, parallel to sta's path
                nc.scalar.copy(stb, ps_av_b[0:65, :])
                nc.scalar.dma_start(out=st_e[2 * t + 1][:, :], in_=stb)

            # ---------------- K^T proj + pair 0, chunk-pipelined ----------
            # K2T[d, j]: K^T computed twice via col-tiled dual matmul groups
            # (cols 0:64 / 64:128 run concurrently) -> one [128, 512] psum.
            # V projection is emitted inside pair 0's j loop so the PE
            # stream stays dense while exps drain.
            kps = {}

            def emit_k_part(nj, fts, done, cs=None, ks=None, dup=True):
                cs = cs if cs is not None else slice(nj * 512, (nj + 1) * 512)
                if nj not in kps:
                    kps[nj] = PS.tile([128, 512], f32, tag="av", name="ps_k", bufs=4)
                ps_k = kps[nj]
                if ks is not None:
                    ps_k = ps_k[:, ks]
                for ft in fts:
                    nc.tensor.matmul(
                        ps_k[0:64, :],
                        lhsT=w0s[:, ft * 2 * D : ft * 2 * D + D],
                        rhs=xTs(ft, cs),
                        start=(ft == 0),
                        stop=(ft == FT - 1),
                        skip_group_check=True,
                    )
                    if dup:
                        nc.tensor.matmul(
                            ps_k[64:128, :],
                            lhsT=w0s[:, ft * 2 * D : ft * 2 * D + D],
                            rhs=xTs(ft, cs),
                            start=(ft == 0),
                            stop=(ft == FT - 1),
                            tile_position=(0, 64),
                            skip_group_check=True,
                        )
                if done:
                    src_ps = kps.pop(nj) if ks is None else ps_k
                    if dup:
                        nc.vector.tensor_copy(k2t[:, cs], src_ps)
                    else:
                        nc.vector.tensor_copy(k2t[0:64, cs], src_ps[0:64, :])
                        # head-b S matmuls need K on partitions 64:128 too;
                        # the gpsimd DMA queue is idle until the first output
                        nc.gpsimd.dma_start(
                            out=k2t[64:128, cs], in_=k2t[0:64, cs]
                        )

            def emit_k(nj):
                emit_k_part(nj, range(FT), True)

            # Preload the exp table set (~2.7us) during the DMA wait so the
            # first real exp doesn't pay it.
            warm_act = P.tile([128, 16], bf16, tag="warm_act", name="warm_act")
            nc.scalar.activation(out=warm_act, in_=junk[:, 0:16], func=EXP)

            # First S matmul needs qt[0] (all of chunk 0) but only keys
            # 0:128 of k2t — interleave both projections with chunk 0's
            # three-piece arrival so the PE consumes each ft pair as it
            # lands. The rest of chunk 0's keys and V follow inside pair 0's
            # early iterations.
            for p_ in range(3):
                fts = (2 * p_, 2 * p_ + 1)
                emit_k_part(0, fts, False, cs=slice(0, 256), ks=slice(0, 256))
                emit_q_part(0, fts, p_ == 2)
            nc.vector.tensor_copy(k2t[:, 0:256], kps[0][:, 0:256])

            def pair0_pre(j):
                # finish chunk 0's keys, pace V one tile per j, spread later
                # K chunks across js, and fold Q1 into the stall window
                if j == 0:
                    emit_k_part(
                        0, range(FT), False, cs=slice(256, 512), ks=slice(256, 512)
                    )
                    nc.vector.tensor_copy(k2t[:, 256:512], kps.pop(0)[:, 256:512])
                    emit_v(0)
                if j + 1 < JT:
                    emit_v(j + 1)
                # K-chunk pacing: emit each chunk's matmuls no earlier than
                # its x DMA lands (in-order PE queue — early emission blocks
                # the S stream behind a data wait). All chunks dup via the
                # dual col-tiled matmul: the second col-group is idle during
                # K projection, so the dup costs no wall time and avoids the
                # gpsimd dup-DMA latency.
                if j == 2:
                    emit_k_part(1, range(0, 3), False, dup=True)
                if j == 3:
                    emit_k_part(1, range(3, FT), True, dup=True)
                if j == 5:
                    emit_k_part(2, range(0, 3), False, dup=True)
                if j == 6:
                    emit_k_part(2, range(3, FT), True, dup=True)
                if j == 9:
                    emit_k_part(3, range(0, 3), False, dup=True)
                if j == 10:
                    emit_k_part(3, range(3, FT), True, dup=True)
                if j == 12:
                    emit_q_part(1, range(3), False)
                if j == 14:
                    emit_q_part(1, range(3, FT), True)

            emit_pair_seg(0, 0, JT, pre=pair0_pre)

            # pair 0 is PE-oversubscribed (all projections + V live there),
            # so its last AV matmuls and output drain spill into pair 1's
            # slack; the av psum pool holds both pairs' accumulators (4
            # bufs) until pair 0 drains at pair-1 j4/j5.
            def pair1_pre(j):
                if j <= 4:
                    emit_av(0, 11 + j, 0)
                if j <= 5:
                    emit_av(0, 10 + j, 1)
                if j == 4:
                    sta = W.tile([65, NQ], bf16, tag="sta", name="sta", bufs=3)
                    nc.scalar.copy(sta, avps[0][0][0:65, :])
                    nc.gpsimd.dma_start(out=st_e[0][:, :], in_=sta)
                if j == 5:
                    stb = W.tile([65, NQ], bf16, tag="stb", name="stb", bufs=3)
                    nc.vector.tensor_copy(stb, avps[0][1][0:65, :])
                    nc.sync.dma_start(out=st_e[1][:, :], in_=stb)
                    avps.pop(0)

            emit_pair_seg(1, 0, JT, pre=pair1_pre)

            # ---------------- remaining pairs -----------------------------
            for t in range(2, H // 2):
                emit_pair_seg(t, 0, JT, pre=lambda j, _t=t: emit_spill(_t - 1, j))
            emit_final_tail()

    _split_multi_waits(nc)
    return nc


def make_in_maps(x, Wq, Wkv, Wproj, bproj):

    def image(a, p=128):
        # [G*p, w] -> [p, G*w] SBUF image (block g at columns g*w:(g+1)*w)
        gp, w = a.shape
        return np.ascontiguousarray(
            a.reshape(gp // p, p, w).transpose(1, 0, 2).reshape(p, -1)
        )

    wq_b = image((Wq * SCALE).astype(BF))
    # regroup to ct-major: piece ct = all six 128-row in-chunks of output
    # columns ct*128:(ct+1)*128, contiguous for piecewise DMA
    wq_b = np.ascontiguousarray(
        wq_b.reshape(128, FT, FT, 128).transpose(0, 2, 1, 3).reshape(128, FT * DIM)
    )
    wkv_b = image(Wkv.astype(BF))
    w0_b = np.ascontiguousarray(np.concatenate([wkv_b, wq_b[:, 0:DIM]], axis=1))
    wqr_b = np.ascontiguousarray(wq_b[:, DIM:])

    xTb = [x[b].T.astype(BF) for b in range(B)]

    in_maps = []
    for c in range(NCORES):
        b, q0 = c // 4, (c % 4) * NQ
        xr = np.roll(xTb[b], -q0, axis=1)  # [768, 2048]
        # image with halves outer: [128, half*6144 + ft*1024 + col]
        xi = (
            xr.reshape(FT, 128, 4, 512)
            .transpose(1, 2, 0, 3)
            .reshape(128, FT * N)
        )
        in_maps.append(
            {
                "xT": np.ascontiguousarray(xi),
                "w0": w0_b,
                "wq": wqr_b,
            }
        )
    return in_maps


def assemble_out(results, Wproj, bproj):
    Wp = Wproj.astype(np.float32)
    bp = bproj.astype(np.float32)
    out = np.empty((B, N, DIM), dtype=np.float32)
    for c in range(NCORES):
        b, q0 = c // 4, (c % 4) * NQ
        o = np.empty((NQ, DIM), dtype=np.float32)
        for h in range(H):
            st = results[c][f"st{h}"].astype(np.float32)
            o[:, h * D : (h + 1) * D] = (st[0:D] / st[D : D + 1]).T
        out[b, q0 : q0 + NQ, :] = o @ Wp + bp
    return out


def kernel(x, Wq, Wkv, Wproj, bproj, num_layer=None):
    from concourse.bass_utils import run_bass_kernel_spmd

    x = np.asarray(x, dtype=np.float32)
    Wq = np.asarray(Wq, dtype=np.float32)
    Wkv = np.asarray(Wkv, dtype=np.float32)
    Wproj = np.asarray(Wproj, dtype=np.float32)
    bproj = np.asarray(bproj, dtype=np.float32)

    in_maps = make_in_maps(x, Wq, Wkv, Wproj, bproj)
    nc = build_graph()
    res = run_bass_kernel_spmd(nc, in_maps, core_ids=list(range(NCORES)))
    return assemble_out(res.results, Wproj, bproj)

